# revision 2
# baseline (speedup 1.0000x reference)
"""DetailBranch guided-filter detail extraction - Trainium2 Bass kernel v7.

Math per image (f16 data planes, f32 psum):
  xn    = clip(x*std+mean, 0, 1)
  gray3 = xn0+xn1+xn2 ; mi3 = mg0+mg1+mg2
  mg_c  = box(xn_c)
  cov_c = box(xn_c*gray3)/867 - mg_c*mi3/3
  den_c = box(xn_c^2)/289 + eps - mg_c^2
  a_c   = cov_c/den_c ; b_c = mi3/3 - a_c*mg_c
  out_c = xn_c - box(a_c)*xn_c - box(b_c)

Box = two TensorE band-matmul passes (unit f16 band moving, data f16
stationary, each pass transposes).  15 box planes/image, stream
software-pipelined (stagger 1, images interleaved, per-channel cluster
order g*, then [gg,gi,a,b] per channel).  All 1/289-type scales folded
as exact f32 scalars into psum-draining ops.

Engine placement: PE matmuls; DVE fast tensor_tensor (2x f16) for the
stage-3 critical chain + fused psum stt drains (cov/q/o); ACT psum
scale/bias drains (mg/geps) + gg squares + most pass1 copies; Pool
(GPSIMD, no PSUM access) slack SBUF ops as 0.6-eff stt (u/nsq/w/s).
I/O f16, host casts.
"""

import sys

sys.path.insert(0, "/opt/trn_rl_repo")

import contextlib

import numpy as np

import concourse.bass as bass
import concourse.mybir as mybir
import concourse.tile as tile

from bass_rust import SyncInfo


EXEMPT = {"InstNoOp", "InstEventSemaphore", "InstAllEngineBarrier",
          "InstSemaphoreOp", "InstHalt"}


def fixup_waits(nc, verbose=False):
    for fn in nc.m.functions:
        targets = []
        for blk in fn.blocks:
            for inst in blk.instructions:
                if (
                    type(inst).__name__ not in EXEMPT
                    and inst.sync_info is not None
                    and len(inst.sync_info.on_wait) > 1
                ):
                    targets.append((blk, inst.name, inst.engine, 1))
        if not targets:
            continue
        for k, (blk, tname, eng, lim) in enumerate(targets):
            il = blk.instructions
            idx = next(j for j, x in enumerate(il) if x.name == tname)
            inst = il[idx]
            si = inst.sync_info
            waits = list(si.on_wait)
            evs = [
                mybir.InstEventSemaphore(
                    name=f"EVW{k}-{j}-{tname}", engine=eng, ins=[], outs=[],
                    sync_info=SyncInfo(on_wait=[w], on_update=[]),
                    bass_nofuse=True,
                )
                for j, w in enumerate(waits[:-lim])
            ]
            inst.sync_info = SyncInfo(
                on_wait=waits[-lim:], on_update=list(si.on_update)
            )
            il[idx:idx] = evs
            if verbose:
                print(f"fixup: {tname}({eng}) {len(waits)} waits -> {len(evs)} evsems")
    return nc


R = 8
KK = float((2 * R + 1) ** 2)  # 289
EPS = 1e-3
H = W = 512
NB = 4
F32 = mybir.dt.float32
F16 = mybir.dt.float16
AF = mybir.ActivationFunctionType
ALU = mybir.AluOpType
IMAGENET_MEAN = [0.485, 0.456, 0.406]
IMAGENET_STD = [0.229, 0.224, 0.225]

COL_RANGES = [(max(0, 128 * k - 8), min(512, 128 * k + 136)) for k in range(NB)]

# psum granularity knobs: True = one [128,2048] tile per pass (1 copy/drain
# op), False = two [128,1024] tiles (2 ops, deeper ring)
PA_COARSE = False
PB_COARSE = False


def band_consts():
    i = np.arange(512)
    B = (np.abs(i[:, None] - i[None, :]) <= R).astype(np.float16)
    band1 = np.zeros((NB, 128, 144), np.float16)
    for k, (lo, hi) in enumerate(COL_RANGES):
        band1[k, :, : hi - lo] = B[128 * k : 128 * k + 128, lo:hi]
    return band1


def build_core_kernel(n_img=2):
    nc = bass.Bass()
    x_ext = nc.dram_tensor("x", [n_img, 3, H, W], F16, kind="ExternalInput")
    band_ext = nc.dram_tensor("band", [NB, 128, 144], F16, kind="ExternalInput")
    out_ext = nc.dram_tensor("out", [n_img, 3, H, W], F16, kind="ExternalOutput")

    with contextlib.ExitStack() as ctx:
        tc = ctx.enter_context(tile.TileContext(nc))
        const = ctx.enter_context(tc.tile_pool(name="const", bufs=1))
        sb = ctx.enter_context(tc.tile_pool(name="sb", bufs=1))
        ps = ctx.enter_context(tc.tile_pool(name="ps", bufs=1, space="PSUM"))

        band = []
        for k in range(NB):
            t = const.tile([128, 144], F16, name=f"band{k}")
            nc.sync.dma_start(t[:], band_ext[k, :, :])
            band.append(t)

        TT = nc.vector.tensor_tensor
        TS = nc.vector.tensor_scalar
        STT = nc.vector.scalar_tensor_tensor
        PSTT = nc.gpsimd.scalar_tensor_tensor

        def plane_tile(name, tag, bufs):
            return sb.tile([128, 2048], F16, name=name, tag=tag, bufs=bufs)

        def pass_tiles(name, tag, coarse, bufs):
            """Allocate psum tiles for one pass; returns list of (tile, col0)."""
            if coarse:
                t = ps.tile([128, 2048], F32, name=f"{name}A", tag=tag, bufs=bufs)
                return [(t, 0)]
            return [
                (ps.tile([128, 1024], F32, name=f"{name}{h}", tag=tag, bufs=bufs), 1024 * h)
                for h in range(2)
            ]

        def pass_mm(src, tiles):
            """Band matmuls: src f16 [128,2048] -> psum tiles (raw sums)."""
            for pt, col0 in tiles:
                for ci in range(pt.shape[-1] // 512):
                    c = (col0 // 512) + ci
                    for k in range(NB):
                        lo, hi = COL_RANGES[k]
                        nc.tensor.matmul(
                            pt[:, 512 * ci + lo : 512 * ci + hi],
                            src[:, 512 * k + 128 * c : 512 * k + 128 * c + 128],
                            band[k][:, : hi - lo],
                            start=(k == 0),
                            stop=(k == 3),
                        )

        st = [dict() for _ in range(n_img)]

        # ---- stage 1 both images ----
        for img in range(n_img):
            sfx = f"i{img}"
            xn = []
            for ch in range(3):
                xin = plane_tile(f"xin{sfx}{ch}", "xin", 2)
                for k in range(NB):
                    nc.sync.dma_start(
                        xin[:, 512 * k : 512 * (k + 1)],
                        x_ext[img, ch, 128 * k : 128 * (k + 1), :],
                    )
                TS(xin[:], xin[:],
                   float(IMAGENET_STD[ch]), float(IMAGENET_MEAN[ch]),
                   ALU.mult, ALU.add)
                xnc = plane_tile(f"xn{sfx}{ch}", "xn", 6)
                TS(xnc[:], xin[:], 0.0, 1.0, ALU.max, ALU.min)
                xn.append(xnc)
            gray3 = plane_tile(f"gray3{sfx}", "gray3", 2)
            TT(gray3[:], xn[0][:], xn[1][:], ALU.add)
            TT(gray3[:], gray3[:], xn[2][:], ALU.add)
            st[img].update(xn=xn, gray3=gray3, mg=[None] * 3, geps=[None] * 3,
                           cov=[None] * 3, a=[None] * 3, b=[None] * 3,
                           q=[None] * 3, s=[None] * 3, nsq=[None] * 3,
                           u=[None] * 3)

        def src_of(img, kind, ch):
            S = st[img]
            sfx = f"i{img}{ch}"
            if kind == "g":
                return S["xn"][ch]
            if kind == "gg":
                t = plane_tile(f"gg{sfx}", "prod", 2)
                nc.gpsimd.tensor_mul(t[:], S["xn"][ch][:], S["xn"][ch][:])
                # u_c = (mi3/3)*mg_c on Pool (slack until gi drain)
                u = plane_tile(f"u{img}{ch}", "u", 3)
                TT(u[:], S["mi"][:], S["mg"][ch][:], ALU.mult)
                S["u"][ch] = u
                nsq = plane_tile(f"nsq{img}{ch}", "nsq", 3)
                nc.gpsimd.tensor_mul(nsq[:], S["mg"][ch][:], S["mg"][ch][:])
                S["nsq"][ch] = nsq
                return t
            if kind == "gi":
                t = plane_tile(f"gi{sfx}", "prod", 2)
                TT(t[:], S["xn"][ch][:], S["gray3"][:], ALU.mult)
                return t
            if kind == "a":
                return S["a"][ch]
            if kind == "b":
                return S["b"][ch]
            raise KeyError(kind)

        def drain(img, kind, ch, pb, col0):
            S = st[img]
            sfx = f"i{img}{ch}"
            sl = slice(col0, col0 + pb.shape[-1])
            if kind == "g":
                if S["mg"][ch] is None:
                    S["mg"][ch] = plane_tile(f"mg{sfx}", "mg", 6)
                nc.scalar.activation(S["mg"][ch][:, sl], pb[:], AF.Copy,
                                     scale=1.0 / KK)
            elif kind == "gg":
                if S["geps"][ch] is None:
                    S["geps"][ch] = plane_tile(f"ge{sfx}", "geps", 3)
                nc.scalar.activation(S["geps"][ch][:, sl], pb[:], AF.Copy,
                                     scale=1.0 / KK, bias=EPS)
            elif kind == "gi":
                if S["cov"][ch] is None:
                    S["cov"][ch] = plane_tile(f"cov{sfx}", "cov", 3)
                STT(S["cov"][ch][:, sl], pb[:], 1.0 / (3.0 * KK),
                    S["u"][ch][:, sl], ALU.mult, ALU.subtract)
            elif kind == "a":
                if S["q"][ch] is None:
                    S["q"][ch] = plane_tile(f"q{sfx}", "q", 2)
                STT(S["q"][ch][:, sl], pb[:], 1.0 / KK,
                    S["xn"][ch][:, sl], ALU.mult, ALU.mult)
            elif kind == "b":
                STT(S["s"][ch][:, sl], pb[:], -1.0 / KK,
                    S["s"][ch][:, sl], ALU.mult, ALU.add)

        def after_drain(img, kind, ch):
            S = st[img]
            sfx = f"i{img}{ch}"
            if kind == "g" and ch == 2:
                mi = plane_tile(f"mi{sfx}", "mi", 2)
                TT(mi[:], S["mg"][0][:], S["mg"][1][:], ALU.add)
                TT(mi[:], mi[:], S["mg"][2][:], ALU.add)
                TS(mi[:], mi[:], 1.0 / 3.0, None, ALU.mult)
                S["mi"] = mi
            elif kind == "gi":
                den = plane_tile(f"den{sfx}", "den", 2)
                TT(den[:], S["geps"][ch][:], S["nsq"][ch][:], ALU.subtract)
                nc.scalar.activation(den[:], den[:], AF.Ln)
                nc.scalar.activation(den[:], den[:], AF.Exp, scale=-1.0)
                a = plane_tile(f"a{sfx}", "a", 3)
                TT(a[:], S["cov"][ch][:], den[:], ALU.mult)
                S["a"][ch] = a
                w = plane_tile(f"w{sfx}", "w", 2)
                nc.gpsimd.tensor_mul(w[:], a[:], S["mg"][ch][:])
                b = plane_tile(f"b{sfx}", "b", 3)
                nc.gpsimd.tensor_sub(b[:], S["mi"][:], w[:])
                S["b"][ch] = b
            elif kind == "a":
                s = plane_tile(f"s{sfx}", "s", 3)
                TT(s[:], S["xn"][ch][:], S["q"][ch][:], ALU.subtract)
                S["s"][ch] = s
            elif kind == "b":
                o = S["s"][ch]
                for k in range(NB):
                    nc.sync.dma_start(
                        out_ext[img, ch, 128 * k : 128 * (k + 1), :],
                        o[:, 512 * k : 512 * (k + 1)],
                    )

        kinds = (
            [("g", c) for c in range(3)]
            + [("gg", 0), ("gi", 0), ("gg", 1), ("a", 0), ("gi", 1),
               ("b", 0), ("gg", 2), ("a", 1), ("gi", 2), ("b", 1),
               ("a", 2), ("b", 2)]
        )
        planes = [(img, kind, ch) for (kind, ch) in kinds for img in range(n_img)]

        # copy engines per plane kind: early phase (g) -> DVE has slack;
        # cluster phase -> ACT
        def copy_engines(idx, kind):
            if kind in ("g",):
                return ("act", "dve")
            if kind == "gg":
                return ("act", "dve") if idx % 2 == 0 else ("act", "act")
            return ("act", "act")

        def emit_copy(vt, pa_tiles, engs):
            for (pt, col0), eng in zip(pa_tiles, engs):
                dst = vt[:, col0 : col0 + pt.shape[-1]]
                if eng == "act":
                    nc.scalar.activation(dst, pt[:], AF.Copy)
                else:
                    nc.vector.tensor_copy(dst, pt[:])

        vt_of = {}
        prev = None
        pa_bufs = 1 if PA_COARSE else 2
        pb_bufs = 1 if PB_COARSE else 2
        for idx, (img, kind, ch) in enumerate(planes):
            name = f"{kind}{img}{ch}"
            src = src_of(img, kind, ch)
            pa = pass_tiles(f"pa_{name}", "pA", PA_COARSE, pa_bufs)
            pass_mm(src, pa)
            vt = plane_tile(f"vt_{name}", "vt", 3)
            emit_copy(vt, pa, copy_engines(idx, kind))
            vt_of[(img, kind, ch)] = vt

            if prev is not None:
                pimg, pkind, pch = prev
                pvt = vt_of.pop(prev)
                pb = pass_tiles(f"pb_{pkind}{pimg}{pch}", "pB", PB_COARSE, pb_bufs)
                pass_mm(pvt, pb)
                for pt, col0 in pb:
                    drain(pimg, pkind, pch, pt, col0)
                after_drain(pimg, pkind, pch)
            prev = (img, kind, ch)

        pimg, pkind, pch = prev
        pvt = vt_of.pop(prev)
        pb = pass_tiles(f"pb_{pkind}{pimg}{pch}", "pB", PB_COARSE, pb_bufs)
        pass_mm(pvt, pb)
        for pt, col0 in pb:
            drain(pimg, pkind, pch, pt, col0)
        after_drain(pimg, pkind, pch)

    fixup_waits(nc)
    return nc


_CACHED = {}


def _get_nc():
    if "nc" not in _CACHED:
        _CACHED["nc"] = build_core_kernel()
    return _CACHED["nc"]


def kernel(x: np.ndarray) -> np.ndarray:
    from concourse.bass_utils import run_bass_kernel_spmd

    assert x.shape == (16, 3, 512, 512)
    band = band_consts()
    x16 = x.astype(np.float16)
    nc = _get_nc()
    in_maps = [
        {
            "x": np.ascontiguousarray(x16[2 * i : 2 * i + 2]),
            "band": band,
        }
        for i in range(8)
    ]
    res = run_bass_kernel_spmd(nc, in_maps, core_ids=list(range(8)))
    return np.concatenate(
        [np.asarray(r["out"]).astype(np.float32) for r in res.results], axis=0
    )


if __name__ == "__main__":
    x = np.random.default_rng(0).standard_normal((16, 3, 512, 512)).astype(np.float32)
    y = kernel(x)
    print(y.shape, y.dtype, float(np.abs(y).max()))


# revision 4
# speedup vs baseline: 1.0327x; 1.0327x over previous
"""DetailBranch (guided-filter detail extraction) Trainium2 Bass kernel.

r=8 -> 17x17 zero-padded box mean, eps=1e-3.  Math per image (f16 data
planes, f32 psum accumulate):
  xn    = clip(x*std+mean, 0, 1)
  gray3 = xn0+xn1+xn2 ; mi = (mg0+mg1+mg2)/3
  mg_c  = box(xn_c)
  cov_c = box(xn_c*gray3)/867 - mg_c*mi      (= cov(g, gray))
  den_c = box(xn_c^2)/289 + eps - mg_c^2     (= var + eps)
  a_c   = cov_c * exp(-ln(den_c)) ; b_c = mi - a_c*mg_c
  out_c = xn_c - box(a_c)*xn_c - box(b_c)

Box = two TensorE band-matmul passes; each filters the partition dim and
transposes (two passes -> upright).  The moving operand is a unit-valued
f16 band matrix (exact; 1 cycle/row), the stationary operand is the f16
data plane; all 1/289-style normalizations are folded into the
psum-draining elementwise ops as exact f32 scalars.

15 box planes per image (g,gi,gg,a,b x3 channels), both images
interleaved plane-by-plane, per-channel clusters staggered
(gg0,gi0,gg1,a0,gi1,b0,...) so each stage-3 chain has pipeline slack.
Stream is software-pipelined with stagger 1 over two psum rings
(pass1: 2x[128,1024], pass2: 2x[128,1024] = all 8 banks).

Engine placement (GPSIMD cannot access PSUM; TensorScalarPtr and
tensor-tensor-divide are invalid on Pool/DVE respectively, hence
exp(-ln) on ACT for the reciprocal):
  PE   band matmuls (f16, cost ~ moving rows only)
  DVE  2x-mode f16 tensor_tensor chain ops + fused psum stt drains
       (cov/q/o), some pass1 copies
  ACT  psum scale/bias drains (mg/geps), Ln/Exp, most pass1 copies
  Pool SBUF-only muls/subs (gg, nsq, w, b)
I/O is f16 (host casts f32<->f16); one DMA per input plane, per-half
output DMAs via 3-d DRAM access patterns.

Sharding: pure batch data-parallel, 2 images per core on 8 cores.
"""

import sys

sys.path.insert(0, "/opt/trn_rl_repo")

import contextlib
import dataclasses

import numpy as np

import concourse.bass as bass
import concourse.mybir as mybir
import concourse.tile as tile

from bass_rust import SyncInfo


EXEMPT = {"InstNoOp", "InstEventSemaphore", "InstAllEngineBarrier",
          "InstSemaphoreOp", "InstHalt"}


def fixup_waits(nc, verbose=False):
    for fn in nc.m.functions:
        targets = []
        for blk in fn.blocks:
            for inst in blk.instructions:
                if (
                    type(inst).__name__ not in EXEMPT
                    and inst.sync_info is not None
                    and len(inst.sync_info.on_wait) > 1
                ):
                    targets.append((blk, inst.name, inst.engine, 1))
        if not targets:
            continue
        for k, (blk, tname, eng, lim) in enumerate(targets):
            il = blk.instructions
            idx = next(j for j, x in enumerate(il) if x.name == tname)
            inst = il[idx]
            si = inst.sync_info
            waits = list(si.on_wait)
            evs = [
                mybir.InstEventSemaphore(
                    name=f"EVW{k}-{j}-{tname}", engine=eng, ins=[], outs=[],
                    sync_info=SyncInfo(on_wait=[w], on_update=[]),
                    bass_nofuse=True,
                )
                for j, w in enumerate(waits[:-lim])
            ]
            inst.sync_info = SyncInfo(
                on_wait=waits[-lim:], on_update=list(si.on_update)
            )
            il[idx:idx] = evs
            if verbose:
                print(f"fixup: {tname}({eng}) {len(waits)} waits -> {len(evs)} evsems")
    return nc


R = 8
KK = float((2 * R + 1) ** 2)  # 289
EPS = 1e-3
H = W = 512
NB = 4
F32 = mybir.dt.float32
F16 = mybir.dt.float16
AF = mybir.ActivationFunctionType
ALU = mybir.AluOpType
IMAGENET_MEAN = [0.485, 0.456, 0.406]
IMAGENET_STD = [0.229, 0.224, 0.225]

COL_RANGES = [(max(0, 128 * k - 8), min(512, 128 * k + 136)) for k in range(NB)]

# psum granularity knobs: True = one [128,2048] tile per pass (1 copy/drain
# op), False = two [128,1024] tiles (2 ops, deeper ring)
PA_COARSE = False
PB_COARSE = False


def band_consts():
    i = np.arange(512)
    B = (np.abs(i[:, None] - i[None, :]) <= R).astype(np.float16)
    band1 = np.zeros((NB, 128, 144), np.float16)
    for k, (lo, hi) in enumerate(COL_RANGES):
        band1[k, :, : hi - lo] = B[128 * k : 128 * k + 128, lo:hi]
    return band1


def build_core_kernel(n_img=2):
    nc = bass.Bass()
    x_ext = nc.dram_tensor("x", [n_img, 3, H, W], F16, kind="ExternalInput")
    band_ext = nc.dram_tensor("band", [NB, 128, 144], F16, kind="ExternalInput")
    out_ext = nc.dram_tensor("out", [n_img, 3, H, W], F16, kind="ExternalOutput")

    with contextlib.ExitStack() as ctx:
        tc = ctx.enter_context(tile.TileContext(nc))
        const = ctx.enter_context(tc.tile_pool(name="const", bufs=1))
        sb = ctx.enter_context(tc.tile_pool(name="sb", bufs=1))
        ps = ctx.enter_context(tc.tile_pool(name="ps", bufs=1, space="PSUM"))

        def dram_plane_ap(ext, img, ch, nb=NB):
            ap = ext[img, ch, 0:128, :]
            return dataclasses.replace(
                ap, ap=mybir.VecI64Pair([[512, 128], [65536, nb], [1, 512]])
            )

        bandt = const.tile([128, NB * 144], F16, name="band")
        band = [bandt[:, 144 * k : 144 * (k + 1)] for k in range(NB)]

        TT = nc.vector.tensor_tensor
        TS = nc.vector.tensor_scalar
        STT = nc.vector.scalar_tensor_tensor
        PSTT = nc.gpsimd.scalar_tensor_tensor

        def plane_tile(name, tag, bufs):
            return sb.tile([128, 2048], F16, name=name, tag=tag, bufs=bufs)

        def pass_tiles(name, tag, coarse, bufs):
            """Allocate psum tiles for one pass; returns list of (tile, col0)."""
            if coarse:
                t = ps.tile([128, 2048], F32, name=f"{name}A", tag=tag, bufs=bufs)
                return [(t, 0)]
            return [
                (ps.tile([128, 1024], F32, name=f"{name}{h}", tag=tag, bufs=bufs), 1024 * h)
                for h in range(2)
            ]

        def pass_mm(src, tiles):
            """Band matmuls: src f16 [128,2048] -> psum tiles (raw sums)."""
            for pt, col0 in tiles:
                for ci in range(pt.shape[-1] // 512):
                    c = (col0 // 512) + ci
                    for k in range(NB):
                        lo, hi = COL_RANGES[k]
                        nc.tensor.matmul(
                            pt[:, 512 * ci + lo : 512 * ci + hi],
                            src[:, 512 * k + 128 * c : 512 * k + 128 * c + 128],
                            band[k][:, : hi - lo],
                            start=(k == 0),
                            stop=(k == 3),
                        )

        st = [dict() for _ in range(n_img)]

        # ---- stage 1 both images ----
        for img in range(n_img):
            sfx = f"i{img}"
            xn = []
            for ch in range(3):
                xin = plane_tile(f"xin{sfx}{ch}", "xin", 2)
                nc.sync.dma_start(xin[:], dram_plane_ap(x_ext, img, ch))
                if img == 0 and ch == 0:
                    # band DMA after the first input plane (head-latency)
                    bext = dataclasses.replace(
                        band_ext[0, 0:128, :],
                        ap=mybir.VecI64Pair([[144, 128], [128 * 144, NB], [1, 144]]),
                    )
                    nc.sync.dma_start(bandt[:], bext)
                TS(xin[:], xin[:],
                   float(IMAGENET_STD[ch]), float(IMAGENET_MEAN[ch]),
                   ALU.mult, ALU.add)
                xnc = plane_tile(f"xn{sfx}{ch}", "xn", 6)
                TS(xnc[:], xin[:], 0.0, 1.0, ALU.max, ALU.min)
                xn.append(xnc)
            gray3 = plane_tile(f"gray3{sfx}", "gray3", 2)
            TT(gray3[:], xn[0][:], xn[1][:], ALU.add)
            TT(gray3[:], gray3[:], xn[2][:], ALU.add)
            st[img].update(xn=xn, gray3=gray3, mg=[None] * 3, geps=[None] * 3,
                           cov=[None] * 3, a=[None] * 3, b=[None] * 3,
                           q=[None] * 3, s=[None] * 3, nsq=[None] * 3,
                           u=[None] * 3, den=[None] * 3, w=[None] * 3)

        def src_of(img, kind, ch):
            S = st[img]
            sfx = f"i{img}{ch}"
            if kind == "g":
                return S["xn"][ch]
            if kind == "gg":
                t = plane_tile(f"gg{sfx}", "prod", 2)
                nc.gpsimd.tensor_mul(t[:], S["xn"][ch][:], S["xn"][ch][:])
                # u_c = (mi3/3)*mg_c on Pool (slack until gi drain)
                u = plane_tile(f"u{img}{ch}", "u", 3)
                TT(u[:], S["mi"][:], S["mg"][ch][:], ALU.mult)
                S["u"][ch] = u
                nsq = plane_tile(f"nsq{img}{ch}", "nsq", 3)
                nc.gpsimd.tensor_mul(nsq[:], S["mg"][ch][:], S["mg"][ch][:])
                S["nsq"][ch] = nsq
                return t
            if kind == "gi":
                t = plane_tile(f"gi{sfx}", "prod", 2)
                TT(t[:], S["xn"][ch][:], S["gray3"][:], ALU.mult)
                return t
            if kind == "a":
                return S["a"][ch]
            if kind == "b":
                return S["b"][ch]
            raise KeyError(kind)

        def drain(img, kind, ch, pb, col0):
            S = st[img]
            sfx = f"i{img}{ch}"
            sl = slice(col0, col0 + pb.shape[-1])
            if kind == "g":
                if S["mg"][ch] is None:
                    S["mg"][ch] = plane_tile(f"mg{sfx}", "mg", 6)
                nc.scalar.activation(S["mg"][ch][:, sl], pb[:], AF.Copy,
                                     scale=1.0 / KK)
            elif kind == "gg":
                if S["geps"][ch] is None:
                    S["geps"][ch] = plane_tile(f"ge{sfx}", "geps", 3)
                nc.scalar.activation(S["geps"][ch][:, sl], pb[:], AF.Copy,
                                     scale=1.0 / KK, bias=EPS)
            elif kind == "gi":
                if S["cov"][ch] is None:
                    S["cov"][ch] = plane_tile(f"cov{sfx}", "cov", 3)
                    S["den"][ch] = plane_tile(f"dn{sfx}", "den", 2)
                    S["a"][ch] = plane_tile(f"a{sfx}", "a", 3)
                    S["w"][ch] = plane_tile(f"w{sfx}", "w", 2)
                    S["b"][ch] = plane_tile(f"b{sfx}", "b", 3)
                cov = S["cov"][ch]
                den = S["den"][ch]
                a = S["a"][ch]
                w = S["w"][ch]
                b = S["b"][ch]
                STT(cov[:, sl], pb[:], 1.0 / (3.0 * KK),
                    S["u"][ch][:, sl], ALU.mult, ALU.subtract)
                TT(den[:, sl], S["geps"][ch][:, sl], S["nsq"][ch][:, sl],
                   ALU.subtract)
                nc.scalar.activation(den[:, sl], den[:, sl], AF.Ln)
                nc.scalar.activation(den[:, sl], den[:, sl], AF.Exp, scale=-1.0)
                TT(a[:, sl], cov[:, sl], den[:, sl], ALU.mult)
                nc.gpsimd.tensor_mul(w[:, sl], a[:, sl], S["mg"][ch][:, sl])
                nc.gpsimd.tensor_sub(b[:, sl], S["mi"][:, sl], w[:, sl])
            elif kind == "a":
                if S["q"][ch] is None:
                    S["q"][ch] = plane_tile(f"q{sfx}", "q", 2)
                    S["s"][ch] = plane_tile(f"s{sfx}", "s", 3)
                STT(S["q"][ch][:, sl], pb[:], 1.0 / KK,
                    S["xn"][ch][:, sl], ALU.mult, ALU.mult)
                TT(S["s"][ch][:, sl], S["xn"][ch][:, sl], S["q"][ch][:, sl],
                   ALU.subtract)
            elif kind == "b":
                STT(S["s"][ch][:, sl], pb[:], -1.0 / KK,
                    S["s"][ch][:, sl], ALU.mult, ALU.add)
                h0 = col0 // 1024
                oap = dataclasses.replace(
                    out_ext[img, ch, 256 * h0 : 256 * h0 + 128, :],
                    ap=mybir.VecI64Pair([[512, 128], [65536, 2], [1, 512]]),
                )
                nc.sync.dma_start(oap, S["s"][ch][:, sl])

        def after_drain(img, kind, ch):
            S = st[img]
            sfx = f"i{img}{ch}"
            if kind == "g" and ch == 2:
                mi = plane_tile(f"mi{sfx}", "mi", 2)
                TT(mi[:], S["mg"][0][:], S["mg"][1][:], ALU.add)
                TT(mi[:], mi[:], S["mg"][2][:], ALU.add)
                TS(mi[:], mi[:], 1.0 / 3.0, None, ALU.mult)
                S["mi"] = mi
            elif kind == "gi":
                pass
            elif kind == "a":
                pass
            elif kind == "b":
                pass

        kinds = (
            [("g", c) for c in range(3)]
            + [("gg", 0), ("gi", 0), ("gg", 1), ("a", 0), ("gi", 1),
               ("b", 0), ("gg", 2), ("a", 1), ("gi", 2), ("b", 1),
               ("a", 2), ("b", 2)]
        )
        planes = [(img, kind, ch) for (kind, ch) in kinds for img in range(n_img)]

        # copy engines per plane kind: early phase (g) -> DVE has slack;
        # cluster phase -> ACT
        def copy_engines(idx, kind):
            if kind in ("g",):
                return ("act", "dve")
            if kind == "gg":
                return ("act", "act")
            return ("act", "act")

        def emit_copy(vt, pa_tiles, engs):
            for (pt, col0), eng in zip(pa_tiles, engs):
                dst = vt[:, col0 : col0 + pt.shape[-1]]
                if eng == "act":
                    nc.scalar.activation(dst, pt[:], AF.Copy)
                else:
                    nc.vector.tensor_copy(dst, pt[:])

        vt_of = {}
        prev = None
        pa_bufs = 1 if PA_COARSE else 2
        pb_bufs = 1 if PB_COARSE else 2
        for idx, (img, kind, ch) in enumerate(planes):
            name = f"{kind}{img}{ch}"
            src = src_of(img, kind, ch)
            pa = pass_tiles(f"pa_{name}", "pA", PA_COARSE, pa_bufs)
            pass_mm(src, pa)
            vt = plane_tile(f"vt_{name}", "vt", 3)
            emit_copy(vt, pa, copy_engines(idx, kind))
            vt_of[(img, kind, ch)] = vt

            if prev is not None:
                pimg, pkind, pch = prev
                pvt = vt_of.pop(prev)
                pb = pass_tiles(f"pb_{pkind}{pimg}{pch}", "pB", PB_COARSE, pb_bufs)
                pass_mm(pvt, pb)
                for pt, col0 in pb:
                    drain(pimg, pkind, pch, pt, col0)
                after_drain(pimg, pkind, pch)
            prev = (img, kind, ch)

        pimg, pkind, pch = prev
        pvt = vt_of.pop(prev)
        pb = pass_tiles(f"pb_{pkind}{pimg}{pch}", "pB", PB_COARSE, pb_bufs)
        pass_mm(pvt, pb)
        for pt, col0 in pb:
            drain(pimg, pkind, pch, pt, col0)
        after_drain(pimg, pkind, pch)

    fixup_waits(nc)
    return nc


_CACHED = {}


def _get_nc():
    if "nc" not in _CACHED:
        _CACHED["nc"] = build_core_kernel()
    return _CACHED["nc"]


def kernel(x: np.ndarray) -> np.ndarray:
    from concourse.bass_utils import run_bass_kernel_spmd

    assert x.shape == (16, 3, 512, 512)
    band = band_consts()
    x16 = x.astype(np.float16)
    nc = _get_nc()
    in_maps = [
        {
            "x": np.ascontiguousarray(x16[2 * i : 2 * i + 2]),
            "band": band,
        }
        for i in range(8)
    ]
    res = run_bass_kernel_spmd(nc, in_maps, core_ids=list(range(8)))
    return np.concatenate(
        [np.asarray(r["out"]).astype(np.float32) for r in res.results], axis=0
    )


if __name__ == "__main__":
    x = np.random.default_rng(0).standard_normal((16, 3, 512, 512)).astype(np.float32)
    y = kernel(x)
    print(y.shape, y.dtype, float(np.abs(y).max()))


# revision 5
# speedup vs baseline: 1.0410x; 1.0080x over previous
"""DetailBranch (guided-filter detail extraction) Trainium2 Bass kernel.

r=8 -> 17x17 zero-padded box mean, eps=1e-3.  Math per image (f16 data
planes, f32 psum accumulate):
  xn    = clip(x*std+mean, 0, 1)
  gray3 = xn0+xn1+xn2 ; mi = (mg0+mg1+mg2)/3
  mg_c  = box(xn_c)
  cov_c = box(xn_c*gray3)/867 - mg_c*mi      (= cov(g, gray))
  den_c = box(xn_c^2)/289 + eps - mg_c^2     (= var + eps)
  a_c   = cov_c * exp(-ln(den_c)) ; b_c = mi - a_c*mg_c
  out_c = xn_c - box(a_c)*xn_c - box(b_c)

Box = two TensorE band-matmul passes; each filters the partition dim and
transposes (two passes -> upright).  The moving operand is a unit-valued
f16 band matrix (exact; 1 cycle/row), the stationary operand is the f16
data plane; all 1/289-style normalizations are folded into the
psum-draining elementwise ops as exact f32 scalars.

15 box planes per image (g,gi,gg,a,b x3 channels), both images
interleaved plane-by-plane, per-channel clusters staggered
(gg0,gi0,gg1,a0,gi1,b0,...) so each stage-3 chain has pipeline slack.
Stream is software-pipelined with stagger 1 over two psum rings
(pass1: 2x[128,1024], pass2: 2x[128,1024] = all 8 banks).

Engine placement (GPSIMD cannot access PSUM; TensorScalarPtr and
tensor-tensor-divide are invalid on Pool/DVE respectively, hence
exp(-ln) on ACT for the reciprocal):
  PE   band matmuls (f16, cost ~ moving rows only)
  DVE  2x-mode f16 tensor_tensor chain ops + fused psum stt drains
       (cov/q/o), some pass1 copies
  ACT  psum scale/bias drains (mg/geps), Ln/Exp, most pass1 copies
  Pool SBUF-only muls/subs (gg, nsq, w, b)
I/O is f16 (host casts f32<->f16); one DMA per input plane, per-half
output DMAs via 3-d DRAM access patterns.  16 dummy band matmuls warm
the PE p-state during the stage-1 DMA latency (full clock needs ~3us of
continuous PE busy).

Sharding: pure batch data-parallel, 2 images per core on 8 cores.
"""

import sys

sys.path.insert(0, "/opt/trn_rl_repo")

import contextlib
import dataclasses

import numpy as np

import concourse.bass as bass
import concourse.mybir as mybir
import concourse.tile as tile

from bass_rust import SyncInfo


EXEMPT = {"InstNoOp", "InstEventSemaphore", "InstAllEngineBarrier",
          "InstSemaphoreOp", "InstHalt"}


def fixup_waits(nc, verbose=False):
    for fn in nc.m.functions:
        targets = []
        for blk in fn.blocks:
            for inst in blk.instructions:
                if (
                    type(inst).__name__ not in EXEMPT
                    and inst.sync_info is not None
                    and len(inst.sync_info.on_wait) > 1
                ):
                    targets.append((blk, inst.name, inst.engine, 1))
        if not targets:
            continue
        for k, (blk, tname, eng, lim) in enumerate(targets):
            il = blk.instructions
            idx = next(j for j, x in enumerate(il) if x.name == tname)
            inst = il[idx]
            si = inst.sync_info
            waits = list(si.on_wait)
            evs = [
                mybir.InstEventSemaphore(
                    name=f"EVW{k}-{j}-{tname}", engine=eng, ins=[], outs=[],
                    sync_info=SyncInfo(on_wait=[w], on_update=[]),
                    bass_nofuse=True,
                )
                for j, w in enumerate(waits[:-lim])
            ]
            inst.sync_info = SyncInfo(
                on_wait=waits[-lim:], on_update=list(si.on_update)
            )
            il[idx:idx] = evs
            if verbose:
                print(f"fixup: {tname}({eng}) {len(waits)} waits -> {len(evs)} evsems")
    return nc


R = 8
KK = float((2 * R + 1) ** 2)  # 289
EPS = 1e-3
H = W = 512
NB = 4
F32 = mybir.dt.float32
F16 = mybir.dt.float16
AF = mybir.ActivationFunctionType
ALU = mybir.AluOpType
IMAGENET_MEAN = [0.485, 0.456, 0.406]
IMAGENET_STD = [0.229, 0.224, 0.225]

COL_RANGES = [(max(0, 128 * k - 8), min(512, 128 * k + 136)) for k in range(NB)]

# psum granularity knobs: True = one [128,2048] tile per pass (1 copy/drain
# op), False = two [128,1024] tiles (2 ops, deeper ring)
PA_COARSE = False
PB_COARSE = False


def band_consts():
    i = np.arange(512)
    B = (np.abs(i[:, None] - i[None, :]) <= R).astype(np.float16)
    band1 = np.zeros((NB, 128, 144), np.float16)
    for k, (lo, hi) in enumerate(COL_RANGES):
        band1[k, :, : hi - lo] = B[128 * k : 128 * k + 128, lo:hi]
    return band1


def build_core_kernel(n_img=2):
    nc = bass.Bass()
    x_ext = nc.dram_tensor("x", [n_img, 3, H, W], F16, kind="ExternalInput")
    band_ext = nc.dram_tensor("band", [NB, 128, 144], F16, kind="ExternalInput")
    out_ext = nc.dram_tensor("out", [n_img, 3, H, W], F16, kind="ExternalOutput")

    with contextlib.ExitStack() as ctx:
        tc = ctx.enter_context(tile.TileContext(nc))
        const = ctx.enter_context(tc.tile_pool(name="const", bufs=1))
        sb = ctx.enter_context(tc.tile_pool(name="sb", bufs=1))
        ps = ctx.enter_context(tc.tile_pool(name="ps", bufs=1, space="PSUM"))

        def dram_plane_ap(ext, img, ch, nb=NB):
            ap = ext[img, ch, 0:128, :]
            return dataclasses.replace(
                ap, ap=mybir.VecI64Pair([[512, 128], [65536, nb], [1, 512]])
            )

        bandt = const.tile([128, NB * 144], F16, name="band")
        band = [bandt[:, 144 * k : 144 * (k + 1)] for k in range(NB)]

        TT = nc.vector.tensor_tensor
        TS = nc.vector.tensor_scalar
        STT = nc.vector.scalar_tensor_tensor
        PSTT = nc.gpsimd.scalar_tensor_tensor

        def plane_tile(name, tag, bufs):
            return sb.tile([128, 2048], F16, name=name, tag=tag, bufs=bufs)

        def pass_tiles(name, tag, coarse, bufs):
            """Allocate psum tiles for one pass; returns list of (tile, col0)."""
            if coarse:
                t = ps.tile([128, 2048], F32, name=f"{name}A", tag=tag, bufs=bufs)
                return [(t, 0)]
            return [
                (ps.tile([128, 1024], F32, name=f"{name}{h}", tag=tag, bufs=bufs), 1024 * h)
                for h in range(2)
            ]

        def pass_mm(src, tiles):
            """Band matmuls: src f16 [128,2048] -> psum tiles (raw sums)."""
            for pt, col0 in tiles:
                for ci in range(pt.shape[-1] // 512):
                    c = (col0 // 512) + ci
                    for k in range(NB):
                        lo, hi = COL_RANGES[k]
                        nc.tensor.matmul(
                            pt[:, 512 * ci + lo : 512 * ci + hi],
                            src[:, 512 * k + 128 * c : 512 * k + 128 * c + 128],
                            band[k][:, : hi - lo],
                            start=(k == 0),
                            stop=(k == 3),
                        )

        warm = ps.tile([128, 1024], F32, name="warm", tag="pA", bufs=2)
        for wi in range(16):
            nc.tensor.matmul(
                warm[:, 0:144], bandt[:, 0:128], bandt[:, 0:144],
                start=True, stop=True,
            )

        st = [dict() for _ in range(n_img)]

        # ---- stage 1 both images ----
        for img in range(n_img):
            sfx = f"i{img}"
            xn = []
            for ch in range(3):
                xin = plane_tile(f"xin{sfx}{ch}", "xin", 2)
                nc.sync.dma_start(xin[:], dram_plane_ap(x_ext, img, ch))
                if img == 0 and ch == 0:
                    # band DMA after the first input plane (head-latency)
                    bext = dataclasses.replace(
                        band_ext[0, 0:128, :],
                        ap=mybir.VecI64Pair([[144, 128], [128 * 144, NB], [1, 144]]),
                    )
                    nc.sync.dma_start(bandt[:], bext)
                TS(xin[:], xin[:],
                   float(IMAGENET_STD[ch]), float(IMAGENET_MEAN[ch]),
                   ALU.mult, ALU.add)
                xnc = plane_tile(f"xn{sfx}{ch}", "xn", 6)
                TS(xnc[:], xin[:], 0.0, 1.0, ALU.max, ALU.min)
                xn.append(xnc)
            gray3 = plane_tile(f"gray3{sfx}", "gray3", 2)
            TT(gray3[:], xn[0][:], xn[1][:], ALU.add)
            TT(gray3[:], gray3[:], xn[2][:], ALU.add)
            st[img].update(xn=xn, gray3=gray3, mg=[None] * 3, geps=[None] * 3,
                           cov=[None] * 3, a=[None] * 3, b=[None] * 3,
                           q=[None] * 3, s=[None] * 3, nsq=[None] * 3,
                           u=[None] * 3, den=[None] * 3, w=[None] * 3)

        def src_of(img, kind, ch):
            S = st[img]
            sfx = f"i{img}{ch}"
            if kind == "g":
                return S["xn"][ch]
            if kind == "gg":
                t = plane_tile(f"gg{sfx}", "prod", 2)
                nc.gpsimd.tensor_mul(t[:], S["xn"][ch][:], S["xn"][ch][:])
                # u_c = (mi3/3)*mg_c on Pool (slack until gi drain)
                u = plane_tile(f"u{img}{ch}", "u", 3)
                TT(u[:], S["mi"][:], S["mg"][ch][:], ALU.mult)
                S["u"][ch] = u
                nsq = plane_tile(f"nsq{img}{ch}", "nsq", 3)
                nc.gpsimd.tensor_mul(nsq[:], S["mg"][ch][:], S["mg"][ch][:])
                S["nsq"][ch] = nsq
                return t
            if kind == "gi":
                t = plane_tile(f"gi{sfx}", "prod", 2)
                TT(t[:], S["xn"][ch][:], S["gray3"][:], ALU.mult)
                return t
            if kind == "a":
                return S["a"][ch]
            if kind == "b":
                return S["b"][ch]
            raise KeyError(kind)

        def drain(img, kind, ch, pb, col0):
            S = st[img]
            sfx = f"i{img}{ch}"
            sl = slice(col0, col0 + pb.shape[-1])
            if kind == "g":
                if S["mg"][ch] is None:
                    S["mg"][ch] = plane_tile(f"mg{sfx}", "mg", 6)
                nc.scalar.activation(S["mg"][ch][:, sl], pb[:], AF.Copy,
                                     scale=1.0 / KK)
            elif kind == "gg":
                if S["geps"][ch] is None:
                    S["geps"][ch] = plane_tile(f"ge{sfx}", "geps", 3)
                nc.scalar.activation(S["geps"][ch][:, sl], pb[:], AF.Copy,
                                     scale=1.0 / KK, bias=EPS)
            elif kind == "gi":
                if S["cov"][ch] is None:
                    S["cov"][ch] = plane_tile(f"cov{sfx}", "cov", 3)
                    S["den"][ch] = plane_tile(f"dn{sfx}", "den", 2)
                    S["a"][ch] = plane_tile(f"a{sfx}", "a", 3)
                    S["w"][ch] = plane_tile(f"w{sfx}", "w", 2)
                    S["b"][ch] = plane_tile(f"b{sfx}", "b", 3)
                cov = S["cov"][ch]
                den = S["den"][ch]
                a = S["a"][ch]
                w = S["w"][ch]
                b = S["b"][ch]
                STT(cov[:, sl], pb[:], 1.0 / (3.0 * KK),
                    S["u"][ch][:, sl], ALU.mult, ALU.subtract)
                TT(den[:, sl], S["geps"][ch][:, sl], S["nsq"][ch][:, sl],
                   ALU.subtract)
                nc.scalar.activation(den[:, sl], den[:, sl], AF.Ln)
                nc.scalar.activation(den[:, sl], den[:, sl], AF.Exp, scale=-1.0)
                TT(a[:, sl], cov[:, sl], den[:, sl], ALU.mult)
                nc.gpsimd.tensor_mul(w[:, sl], a[:, sl], S["mg"][ch][:, sl])
                nc.gpsimd.tensor_sub(b[:, sl], S["mi"][:, sl], w[:, sl])
            elif kind == "a":
                if S["q"][ch] is None:
                    S["q"][ch] = plane_tile(f"q{sfx}", "q", 2)
                    S["s"][ch] = plane_tile(f"s{sfx}", "s", 3)
                STT(S["q"][ch][:, sl], pb[:], 1.0 / KK,
                    S["xn"][ch][:, sl], ALU.mult, ALU.mult)
                TT(S["s"][ch][:, sl], S["xn"][ch][:, sl], S["q"][ch][:, sl],
                   ALU.subtract)
            elif kind == "b":
                STT(S["s"][ch][:, sl], pb[:], -1.0 / KK,
                    S["s"][ch][:, sl], ALU.mult, ALU.add)
                h0 = col0 // 1024
                oap = dataclasses.replace(
                    out_ext[img, ch, 256 * h0 : 256 * h0 + 128, :],
                    ap=mybir.VecI64Pair([[512, 128], [65536, 2], [1, 512]]),
                )
                nc.sync.dma_start(oap, S["s"][ch][:, sl])

        def after_drain(img, kind, ch):
            S = st[img]
            sfx = f"i{img}{ch}"
            if kind == "g" and ch == 2:
                mi = plane_tile(f"mi{sfx}", "mi", 2)
                TT(mi[:], S["mg"][0][:], S["mg"][1][:], ALU.add)
                TT(mi[:], mi[:], S["mg"][2][:], ALU.add)
                TS(mi[:], mi[:], 1.0 / 3.0, None, ALU.mult)
                S["mi"] = mi
            elif kind == "gi":
                pass
            elif kind == "a":
                pass
            elif kind == "b":
                pass

        kinds = (
            [("g", c) for c in range(3)]
            + [("gg", 0), ("gi", 0), ("gg", 1), ("a", 0), ("gi", 1),
               ("b", 0), ("gg", 2), ("a", 1), ("gi", 2), ("b", 1),
               ("a", 2), ("b", 2)]
        )
        planes = [(img, kind, ch) for (kind, ch) in kinds for img in range(n_img)]

        # copy engines per plane kind: early phase (g) -> DVE has slack;
        # cluster phase -> ACT
        def copy_engines(idx, kind):
            if kind in ("g",):
                return ("act", "dve")
            if kind == "gg":
                return ("act", "act")
            return ("act", "act")

        def emit_copy(vt, pa_tiles, engs):
            for (pt, col0), eng in zip(pa_tiles, engs):
                dst = vt[:, col0 : col0 + pt.shape[-1]]
                if eng == "act":
                    nc.scalar.activation(dst, pt[:], AF.Copy)
                else:
                    nc.vector.tensor_copy(dst, pt[:])

        vt_of = {}
        prev = None
        pa_bufs = 1 if PA_COARSE else 2
        pb_bufs = 1 if PB_COARSE else 2
        for idx, (img, kind, ch) in enumerate(planes):
            name = f"{kind}{img}{ch}"
            src = src_of(img, kind, ch)
            pa = pass_tiles(f"pa_{name}", "pA", PA_COARSE, pa_bufs)
            pass_mm(src, pa)
            vt = plane_tile(f"vt_{name}", "vt", 3)
            emit_copy(vt, pa, copy_engines(idx, kind))
            vt_of[(img, kind, ch)] = vt

            if prev is not None:
                pimg, pkind, pch = prev
                pvt = vt_of.pop(prev)
                pb = pass_tiles(f"pb_{pkind}{pimg}{pch}", "pB", PB_COARSE, pb_bufs)
                pass_mm(pvt, pb)
                for pt, col0 in pb:
                    drain(pimg, pkind, pch, pt, col0)
                after_drain(pimg, pkind, pch)
            prev = (img, kind, ch)

        pimg, pkind, pch = prev
        pvt = vt_of.pop(prev)
        pb = pass_tiles(f"pb_{pkind}{pimg}{pch}", "pB", PB_COARSE, pb_bufs)
        pass_mm(pvt, pb)
        for pt, col0 in pb:
            drain(pimg, pkind, pch, pt, col0)
        after_drain(pimg, pkind, pch)

    fixup_waits(nc)
    return nc


_CACHED = {}


def _get_nc():
    if "nc" not in _CACHED:
        _CACHED["nc"] = build_core_kernel()
    return _CACHED["nc"]


def kernel(x: np.ndarray) -> np.ndarray:
    from concourse.bass_utils import run_bass_kernel_spmd

    assert x.shape == (16, 3, 512, 512)
    band = band_consts()
    x16 = x.astype(np.float16)
    nc = _get_nc()
    in_maps = [
        {
            "x": np.ascontiguousarray(x16[2 * i : 2 * i + 2]),
            "band": band,
        }
        for i in range(8)
    ]
    res = run_bass_kernel_spmd(nc, in_maps, core_ids=list(range(8)))
    return np.concatenate(
        [np.asarray(r["out"]).astype(np.float32) for r in res.results], axis=0
    )


if __name__ == "__main__":
    x = np.random.default_rng(0).standard_normal((16, 3, 512, 512)).astype(np.float32)
    y = kernel(x)
    print(y.shape, y.dtype, float(np.abs(y).max()))


# revision 7
# speedup vs baseline: 1.0727x; 1.0304x over previous
"""DetailBranch (guided-filter detail extraction) Trainium2 Bass kernel.

r=8 -> 17x17 zero-padded box mean, eps=1e-3.  Math per image (f16 data
planes, f32 psum accumulate):
  xn    = clip(x*std+mean, 0, 1)
  gray3 = xn0+xn1+xn2 ; mi = (mg0+mg1+mg2)/3
  mg_c  = box(xn_c)
  cov_c = box(xn_c*gray3)/867 - mg_c*mi      (= cov(g, gray))
  den_c = box(xn_c^2)/289 + eps - mg_c^2     (= var + eps)
  a_c   = cov_c * exp(-ln(den_c)) ; b_c = mi - a_c*mg_c
  out_c = xn_c - box(a_c)*xn_c - box(b_c)

Box = two TensorE band-matmul passes; each filters the partition dim and
transposes (two passes -> upright).  The moving operand is a unit-valued
f16 band matrix (exact; 1 cycle/row), the stationary operand is the f16
data plane; all 1/289-style normalizations are folded into the
psum-draining elementwise ops as exact f32 scalars.

15 box planes per image (g,gi,gg,a,b x3 channels), both images
interleaved plane-by-plane, per-channel clusters staggered
(gg0,gi0,gg1,a0,gi1,b0,...) so each stage-3 chain has pipeline slack.
Stream is software-pipelined with stagger 1 over two psum rings
(pass1: 2x[128,1024], pass2: 2x[128,1024] = all 8 banks).

Engine placement (GPSIMD cannot access PSUM; TensorScalarPtr and
tensor-tensor-divide are invalid on Pool/DVE respectively, hence
exp(-ln) on ACT for the reciprocal):
  PE   band matmuls (f16, cost ~ moving rows only)
  DVE  2x-mode f16 tensor_tensor chain ops + fused psum stt drains
       (cov/q/o), some pass1 copies
  ACT  psum scale/bias drains (mg/geps), Ln/Exp, most pass1 copies
  Pool SBUF-only muls/subs (gg, nsq, w, b)
I/O is f16 (host casts f32<->f16); one DMA per input plane, per-half
output DMAs via 3-d DRAM access patterns.  16 dummy band matmuls warm
the PE p-state during the stage-1 DMA latency (full clock needs ~3us of
continuous PE busy).

Sharding: pure batch data-parallel, 2 images per core on 8 cores.
"""

import sys

sys.path.insert(0, "/opt/trn_rl_repo")

import contextlib
import dataclasses

import numpy as np

import concourse.bass as bass
import concourse.mybir as mybir
import concourse.tile as tile

from bass_rust import SyncInfo


EXEMPT = {"InstNoOp", "InstEventSemaphore", "InstAllEngineBarrier",
          "InstSemaphoreOp", "InstHalt"}


def fixup_waits(nc, verbose=False):
    for fn in nc.m.functions:
        targets = []
        for blk in fn.blocks:
            for inst in blk.instructions:
                if (
                    type(inst).__name__ not in EXEMPT
                    and inst.sync_info is not None
                    and len(inst.sync_info.on_wait) > 1
                ):
                    targets.append((blk, inst.name, inst.engine, 1))
        if not targets:
            continue
        for k, (blk, tname, eng, lim) in enumerate(targets):
            il = blk.instructions
            idx = next(j for j, x in enumerate(il) if x.name == tname)
            inst = il[idx]
            si = inst.sync_info
            waits = list(si.on_wait)
            evs = [
                mybir.InstEventSemaphore(
                    name=f"EVW{k}-{j}-{tname}", engine=eng, ins=[], outs=[],
                    sync_info=SyncInfo(on_wait=[w], on_update=[]),
                    bass_nofuse=True,
                )
                for j, w in enumerate(waits[:-lim])
            ]
            inst.sync_info = SyncInfo(
                on_wait=waits[-lim:], on_update=list(si.on_update)
            )
            il[idx:idx] = evs
            if verbose:
                print(f"fixup: {tname}({eng}) {len(waits)} waits -> {len(evs)} evsems")
    return nc


R = 8
KK = float((2 * R + 1) ** 2)  # 289
EPS = 1e-3
H = W = 512
NB = 4
F32 = mybir.dt.float32
F16 = mybir.dt.float16
AF = mybir.ActivationFunctionType
ALU = mybir.AluOpType
IMAGENET_MEAN = [0.485, 0.456, 0.406]
IMAGENET_STD = [0.229, 0.224, 0.225]

COL_RANGES = [(max(0, 128 * k - 8), min(512, 128 * k + 136)) for k in range(NB)]

# psum granularity knobs: True = one [128,2048] tile per pass (1 copy/drain
# op), False = two [128,1024] tiles (2 ops, deeper ring)
PA_COARSE = False
PB_COARSE = False


def band_consts():
    i = np.arange(512)
    B = (np.abs(i[:, None] - i[None, :]) <= R).astype(np.float16)
    band1 = np.zeros((NB, 128, 144), np.float16)
    for k, (lo, hi) in enumerate(COL_RANGES):
        band1[k, :, : hi - lo] = B[128 * k : 128 * k + 128, lo:hi]
    return band1


def build_core_kernel(n_img=2):
    nc = bass.Bass()
    x_ext = nc.dram_tensor("x", [n_img, 3, H, W], F16, kind="ExternalInput")
    band_ext = nc.dram_tensor("band", [NB, 128, 144], F16, kind="ExternalInput")
    out_ext = nc.dram_tensor("out", [n_img, 3, H, W], F16, kind="ExternalOutput")

    with contextlib.ExitStack() as ctx:
        tc = ctx.enter_context(tile.TileContext(nc))
        const = ctx.enter_context(tc.tile_pool(name="const", bufs=1))
        sb = ctx.enter_context(tc.tile_pool(name="sb", bufs=1))
        ps = ctx.enter_context(tc.tile_pool(name="ps", bufs=1, space="PSUM"))

        def dram_plane_ap(ext, img, ch, nb=NB):
            ap = ext[img, ch, 0:128, :]
            return dataclasses.replace(
                ap, ap=mybir.VecI64Pair([[512, 128], [65536, nb], [1, 512]])
            )

        bandt = const.tile([128, NB * 144], F16, name="band")
        band = [bandt[:, 144 * k : 144 * (k + 1)] for k in range(NB)]

        bandn = const.tile([128, NB * 144], F16, name="bandn")
        band_neg = [bandn[:, 144 * k : 144 * (k + 1)] for k in range(NB)]

        TT = nc.vector.tensor_tensor
        TS = nc.vector.tensor_scalar
        STT = nc.vector.scalar_tensor_tensor
        PSTT = nc.gpsimd.scalar_tensor_tensor

        def plane_tile(name, tag, bufs):
            return sb.tile([128, 2048], F16, name=name, tag=tag, bufs=bufs)

        def pass_tiles(name, tag, coarse, bufs):
            """Allocate psum tiles for one pass; returns list of (tile, col0)."""
            if coarse:
                t = ps.tile([128, 2048], F32, name=f"{name}A", tag=tag, bufs=bufs)
                return [(t, 0)]
            return [
                (ps.tile([128, 1024], F32, name=f"{name}{h}", tag=tag, bufs=bufs), 1024 * h)
                for h in range(2)
            ]

        def pass_mm2(src_a, src_b, tiles):
            """pass1 of (src_a - src_b): accumulate band*src_a + (-band)*src_b."""
            for pt, col0 in tiles:
                for ci in range(pt.shape[-1] // 512):
                    c = (col0 // 512) + ci
                    for g2, (srcx, bd) in enumerate(
                        ((src_a, band), (src_b, band_neg))
                    ):
                        for k in range(NB):
                            lo, hi = COL_RANGES[k]
                            nc.tensor.matmul(
                                pt[:, 512 * ci + lo : 512 * ci + hi],
                                srcx[:, 512 * k + 128 * c : 512 * k + 128 * c + 128],
                                bd[k][:, : hi - lo],
                                start=(g2 == 0 and k == 0),
                                stop=(g2 == 1 and k == 3),
                            )

        def pass_mm(src, tiles):
            """Band matmuls: src f16 [128,2048] -> psum tiles (raw sums)."""
            for pt, col0 in tiles:
                for ci in range(pt.shape[-1] // 512):
                    c = (col0 // 512) + ci
                    for k in range(NB):
                        lo, hi = COL_RANGES[k]
                        nc.tensor.matmul(
                            pt[:, 512 * ci + lo : 512 * ci + hi],
                            src[:, 512 * k + 128 * c : 512 * k + 128 * c + 128],
                            band[k][:, : hi - lo],
                            start=(k == 0),
                            stop=(k == 3),
                        )

        warm = ps.tile([128, 1024], F32, name="warm", tag="pA", bufs=2)
        for wi in range(16):
            nc.tensor.matmul(
                warm[:, 0:144], bandt[:, 0:128], bandt[:, 0:144],
                start=True, stop=True,
            )

        st = [dict() for _ in range(n_img)]

        # ---- stage 1 both images ----
        for img in range(n_img):
            sfx = f"i{img}"
            xn = []
            for ch in range(3):
                xin = plane_tile(f"xin{sfx}{ch}", "xin", 2)
                nc.sync.dma_start(xin[:], dram_plane_ap(x_ext, img, ch))
                if img == 0 and ch == 0:
                    # band DMA after the first input plane (head-latency)
                    bext = dataclasses.replace(
                        band_ext[0, 0:128, :],
                        ap=mybir.VecI64Pair([[144, 128], [128 * 144, NB], [1, 144]]),
                    )
                    nc.sync.dma_start(bandt[:], bext)
                    nc.vector.tensor_scalar(bandn[:], bandt[:], -1.0, None,
                                            ALU.mult)
                TS(xin[:], xin[:],
                   float(IMAGENET_STD[ch]), float(IMAGENET_MEAN[ch]),
                   ALU.mult, ALU.add)
                xnc = plane_tile(f"xn{sfx}{ch}", "xn", 6)
                TS(xnc[:], xin[:], 0.0, 1.0, ALU.max, ALU.min)
                xn.append(xnc)
            gray3 = plane_tile(f"gray3{sfx}", "gray3", 2)
            TT(gray3[:], xn[0][:], xn[1][:], ALU.add)
            TT(gray3[:], gray3[:], xn[2][:], ALU.add)
            st[img].update(xn=xn, gray3=gray3, mg=[None] * 3, geps=[None] * 3,
                           cov=[None] * 3, a=[None] * 3, b=[None] * 3,
                           q=[None] * 3, s=[None] * 3, nsq=[None] * 3,
                           u=[None] * 3, den=[None] * 3, w=[None] * 3)

        def src_of(img, kind, ch):
            S = st[img]
            sfx = f"i{img}{ch}"
            if kind == "g":
                return S["xn"][ch]
            if kind == "gg":
                t = plane_tile(f"gg{sfx}", "prod", 2)
                nc.gpsimd.tensor_mul(t[:], S["xn"][ch][:], S["xn"][ch][:])
                # u_c = (mi3/3)*mg_c on Pool (slack until gi drain)
                u = plane_tile(f"u{img}{ch}", "u", 3)
                TT(u[:], S["mi"][:], S["mg"][ch][:], ALU.mult)
                S["u"][ch] = u
                nsq = plane_tile(f"nsq{img}{ch}", "nsq", 3)
                nc.gpsimd.tensor_mul(nsq[:], S["mg"][ch][:], S["mg"][ch][:])
                S["nsq"][ch] = nsq
                return t
            if kind == "gi":
                t = plane_tile(f"gi{sfx}", "prod", 2)
                TT(t[:], S["xn"][ch][:], S["gray3"][:], ALU.mult)
                return t
            if kind == "a":
                return S["a"][ch]
            if kind == "b":
                return (S["mi"], S["w"][ch])
            raise KeyError(kind)

        def drain(img, kind, ch, pb, col0):
            S = st[img]
            sfx = f"i{img}{ch}"
            sl = slice(col0, col0 + pb.shape[-1])
            if kind == "g":
                if S["mg"][ch] is None:
                    S["mg"][ch] = plane_tile(f"mg{sfx}", "mg", 6)
                nc.scalar.activation(S["mg"][ch][:, sl], pb[:], AF.Copy,
                                     scale=1.0 / KK)
            elif kind == "gg":
                if S["geps"][ch] is None:
                    S["geps"][ch] = plane_tile(f"ge{sfx}", "geps", 3)
                nc.scalar.activation(S["geps"][ch][:, sl], pb[:], AF.Copy,
                                     scale=1.0 / KK, bias=EPS)
            elif kind == "gi":
                if S["cov"][ch] is None:
                    S["cov"][ch] = plane_tile(f"cov{sfx}", "cov", 3)
                    S["den"][ch] = plane_tile(f"dn{sfx}", "den", 2)
                    S["a"][ch] = plane_tile(f"a{sfx}", "a", 3)
                    S["w"][ch] = plane_tile(f"w{sfx}", "w", 3)
                cov = S["cov"][ch]
                den = S["den"][ch]
                a = S["a"][ch]
                w = S["w"][ch]
                STT(cov[:, sl], pb[:], 1.0 / (3.0 * KK),
                    S["u"][ch][:, sl], ALU.mult, ALU.subtract)
                TT(den[:, sl], S["geps"][ch][:, sl], S["nsq"][ch][:, sl],
                   ALU.subtract)
                if col0 >= 1024:
                    nc.scalar.activation(den[:], den[:], AF.Ln)
                    nc.scalar.activation(den[:], den[:], AF.Exp, scale=-1.0)
                    for hh in range(2):
                        s2 = slice(1024 * hh, 1024 * (hh + 1))
                        TT(a[:, s2], cov[:, s2], den[:, s2], ALU.mult)
                        nc.gpsimd.tensor_mul(w[:, s2], a[:, s2],
                                             S["mg"][ch][:, s2])
            elif kind == "a":
                if S["q"][ch] is None:
                    S["q"][ch] = plane_tile(f"q{sfx}", "q", 2)
                    S["s"][ch] = plane_tile(f"s{sfx}", "s", 3)
                STT(S["q"][ch][:, sl], pb[:], 1.0 / KK,
                    S["xn"][ch][:, sl], ALU.mult, ALU.mult)
                TT(S["s"][ch][:, sl], S["xn"][ch][:, sl], S["q"][ch][:, sl],
                   ALU.subtract)
            elif kind == "b":
                STT(S["s"][ch][:, sl], pb[:], -1.0 / KK,
                    S["s"][ch][:, sl], ALU.mult, ALU.add)
                h0 = col0 // 1024
                oap = dataclasses.replace(
                    out_ext[img, ch, 256 * h0 : 256 * h0 + 128, :],
                    ap=mybir.VecI64Pair([[512, 128], [65536, 2], [1, 512]]),
                )
                nc.sync.dma_start(oap, S["s"][ch][:, sl])

        def after_drain(img, kind, ch):
            S = st[img]
            sfx = f"i{img}{ch}"
            if kind == "g" and ch == 2:
                mi = plane_tile(f"mi{sfx}", "mi", 2)
                TT(mi[:], S["mg"][0][:], S["mg"][1][:], ALU.add)
                TT(mi[:], mi[:], S["mg"][2][:], ALU.add)
                TS(mi[:], mi[:], 1.0 / 3.0, None, ALU.mult)
                S["mi"] = mi
            elif kind == "gi":
                pass
            elif kind == "a":
                pass
            elif kind == "b":
                pass

        kinds = (
            [("g", c) for c in range(3)]
            + [("gg", 0), ("gi", 0), ("gg", 1), ("a", 0), ("gi", 1),
               ("b", 0), ("gg", 2), ("a", 1), ("gi", 2), ("b", 1),
               ("a", 2), ("b", 2)]
        )
        planes = [(img, kind, ch) for (kind, ch) in kinds for img in range(n_img)]

        # copy engines per plane kind: early phase (g) -> DVE has slack;
        # cluster phase -> ACT
        def copy_engines(idx, kind):
            if kind in ("g",):
                return ("act", "dve")
            if kind == "gg":
                return ("act", "act")
            return ("act", "act")

        def emit_copy(vt, pa_tiles, engs):
            for (pt, col0), eng in zip(pa_tiles, engs):
                dst = vt[:, col0 : col0 + pt.shape[-1]]
                if eng == "act":
                    nc.scalar.activation(dst, pt[:], AF.Copy)
                else:
                    nc.vector.tensor_copy(dst, pt[:])

        vt_of = {}
        prev = None
        pa_bufs = 1 if PA_COARSE else 2
        pb_bufs = 1 if PB_COARSE else 2
        for idx, (img, kind, ch) in enumerate(planes):
            name = f"{kind}{img}{ch}"
            src = src_of(img, kind, ch)
            pa = pass_tiles(f"pa_{name}", "pA", PA_COARSE, pa_bufs)
            if kind == "b":
                pass_mm2(src[0], src[1], pa)
            else:
                pass_mm(src, pa)
            vt = plane_tile(f"vt_{name}", "vt", 3)
            emit_copy(vt, pa, copy_engines(idx, kind))
            vt_of[(img, kind, ch)] = vt

            if prev is not None:
                pimg, pkind, pch = prev
                pvt = vt_of.pop(prev)
                pb = pass_tiles(f"pb_{pkind}{pimg}{pch}", "pB", PB_COARSE, pb_bufs)
                pass_mm(pvt, pb)
                for pt, col0 in pb:
                    drain(pimg, pkind, pch, pt, col0)
                after_drain(pimg, pkind, pch)
            prev = (img, kind, ch)

        pimg, pkind, pch = prev
        pvt = vt_of.pop(prev)
        pb = pass_tiles(f"pb_{pkind}{pimg}{pch}", "pB", PB_COARSE, pb_bufs)
        pass_mm(pvt, pb)
        for pt, col0 in pb:
            drain(pimg, pkind, pch, pt, col0)
        after_drain(pimg, pkind, pch)

    fixup_waits(nc)
    return nc


_CACHED = {}


def _get_nc():
    if "nc" not in _CACHED:
        _CACHED["nc"] = build_core_kernel()
    return _CACHED["nc"]


def kernel(x: np.ndarray) -> np.ndarray:
    from concourse.bass_utils import run_bass_kernel_spmd

    assert x.shape == (16, 3, 512, 512)
    band = band_consts()
    x16 = x.astype(np.float16)
    nc = _get_nc()
    in_maps = [
        {
            "x": np.ascontiguousarray(x16[2 * i : 2 * i + 2]),
            "band": band,
        }
        for i in range(8)
    ]
    res = run_bass_kernel_spmd(nc, in_maps, core_ids=list(range(8)))
    return np.concatenate(
        [np.asarray(r["out"]).astype(np.float32) for r in res.results], axis=0
    )


if __name__ == "__main__":
    x = np.random.default_rng(0).standard_normal((16, 3, 512, 512)).astype(np.float32)
    y = kernel(x)
    print(y.shape, y.dtype, float(np.abs(y).max()))


# revision 8
# speedup vs baseline: 1.0742x; 1.0014x over previous
"""DetailBranch (guided-filter detail extraction) Trainium2 Bass kernel.

r=8 -> 17x17 zero-padded box mean, eps=1e-3.  Math per image (f16 data
planes, f32 psum accumulate):
  xn    = clip(x*std+mean, 0, 1)
  gray3 = xn0+xn1+xn2 ; mi = (mg0+mg1+mg2)/3
  mg_c  = box(xn_c)
  cov_c = box(xn_c*gray3)/867 - mg_c*mi      (= cov(g, gray))
  den_c = box(xn_c^2)/289 + eps - mg_c^2     (= var + eps)
  a_c   = cov_c * exp(-ln(den_c)) ; b_c = mi - a_c*mg_c
  out_c = xn_c - box(a_c)*xn_c - box(b_c)

Box = two TensorE band-matmul passes; each filters the partition dim and
transposes (two passes -> upright).  The moving operand is a unit-valued
f16 band matrix (exact; 1 cycle/row), the stationary operand is the f16
data plane; all 1/289-style normalizations are folded into the
psum-draining elementwise ops as exact f32 scalars.

15 box planes per image (g,gi,gg,a,b x3 channels), both images
interleaved plane-by-plane, per-channel clusters staggered
(gg0,gi0,gg1,a0,gi1,b0,...) so each stage-3 chain has pipeline slack.
Stream is software-pipelined with stagger 1 over two psum rings
(pass1: 2x[128,1024], pass2: 2x[128,1024] = all 8 banks).

Engine placement (GPSIMD cannot access PSUM; TensorScalarPtr and
tensor-tensor-divide are invalid on Pool/DVE respectively, hence
exp(-ln) on ACT for the reciprocal):
  PE   band matmuls (f16, cost ~ moving rows only)
  DVE  2x-mode f16 tensor_tensor chain ops + fused psum stt drains
       (cov/q/o), some pass1 copies
  ACT  psum scale/bias drains (mg/geps), Ln/Exp, most pass1 copies
  Pool SBUF-only muls/subs (gg, nsq, w, b)
I/O is f16 (host casts f32<->f16); one DMA per input plane, per-half
output DMAs via 3-d DRAM access patterns.  16 dummy band matmuls warm
the PE p-state during the stage-1 DMA latency (full clock needs ~3us of
continuous PE busy).

Sharding: pure batch data-parallel, 2 images per core on 8 cores.
"""

import sys

sys.path.insert(0, "/opt/trn_rl_repo")

import contextlib
import dataclasses

import numpy as np

import concourse.bass as bass
import concourse.mybir as mybir
import concourse.tile as tile

from bass_rust import SyncInfo


EXEMPT = {"InstNoOp", "InstEventSemaphore", "InstAllEngineBarrier",
          "InstSemaphoreOp", "InstHalt"}


def fixup_waits(nc, verbose=False):
    for fn in nc.m.functions:
        targets = []
        for blk in fn.blocks:
            for inst in blk.instructions:
                if (
                    type(inst).__name__ not in EXEMPT
                    and inst.sync_info is not None
                    and len(inst.sync_info.on_wait) > 1
                ):
                    targets.append((blk, inst.name, inst.engine, 1))
        if not targets:
            continue
        for k, (blk, tname, eng, lim) in enumerate(targets):
            il = blk.instructions
            idx = next(j for j, x in enumerate(il) if x.name == tname)
            inst = il[idx]
            si = inst.sync_info
            waits = list(si.on_wait)
            evs = [
                mybir.InstEventSemaphore(
                    name=f"EVW{k}-{j}-{tname}", engine=eng, ins=[], outs=[],
                    sync_info=SyncInfo(on_wait=[w], on_update=[]),
                    bass_nofuse=True,
                )
                for j, w in enumerate(waits[:-lim])
            ]
            inst.sync_info = SyncInfo(
                on_wait=waits[-lim:], on_update=list(si.on_update)
            )
            il[idx:idx] = evs
            if verbose:
                print(f"fixup: {tname}({eng}) {len(waits)} waits -> {len(evs)} evsems")
    return nc


R = 8
KK = float((2 * R + 1) ** 2)  # 289
EPS = 1e-3
H = W = 512
NB = 4
F32 = mybir.dt.float32
F16 = mybir.dt.float16
AF = mybir.ActivationFunctionType
ALU = mybir.AluOpType
IMAGENET_MEAN = [0.485, 0.456, 0.406]
IMAGENET_STD = [0.229, 0.224, 0.225]

COL_RANGES = [(max(0, 128 * k - 8), min(512, 128 * k + 136)) for k in range(NB)]

# psum granularity knobs: True = one [128,2048] tile per pass (1 copy/drain
# op), False = two [128,1024] tiles (2 ops, deeper ring)
PA_COARSE = False
PB_COARSE = False


def band_consts():
    i = np.arange(512)
    B = (np.abs(i[:, None] - i[None, :]) <= R).astype(np.float16)
    band1 = np.zeros((NB, 128, 144), np.float16)
    for k, (lo, hi) in enumerate(COL_RANGES):
        band1[k, :, : hi - lo] = B[128 * k : 128 * k + 128, lo:hi]
    return band1


def build_core_kernel(n_img=2):
    nc = bass.Bass()
    x_ext = nc.dram_tensor("x", [n_img, 3, H, W], F16, kind="ExternalInput")
    band_ext = nc.dram_tensor("band", [NB, 128, 144], F16, kind="ExternalInput")
    out_ext = nc.dram_tensor("out", [n_img, 3, H, W], F16, kind="ExternalOutput")

    with contextlib.ExitStack() as ctx:
        tc = ctx.enter_context(tile.TileContext(nc))
        const = ctx.enter_context(tc.tile_pool(name="const", bufs=1))
        sb = ctx.enter_context(tc.tile_pool(name="sb", bufs=1))
        ps = ctx.enter_context(tc.tile_pool(name="ps", bufs=1, space="PSUM"))

        def dram_plane_ap(ext, img, ch, nb=NB):
            ap = ext[img, ch, 0:128, :]
            return dataclasses.replace(
                ap, ap=mybir.VecI64Pair([[512, 128], [65536, nb], [1, 512]])
            )

        bandt = const.tile([128, NB * 144], F16, name="band")
        band = [bandt[:, 144 * k : 144 * (k + 1)] for k in range(NB)]

        bandn = const.tile([128, NB * 144], F16, name="bandn")
        band_neg = [bandn[:, 144 * k : 144 * (k + 1)] for k in range(NB)]

        TT = nc.vector.tensor_tensor
        TS = nc.vector.tensor_scalar
        STT = nc.vector.scalar_tensor_tensor
        PSTT = nc.gpsimd.scalar_tensor_tensor

        def plane_tile(name, tag, bufs):
            return sb.tile([128, 2048], F16, name=name, tag=tag, bufs=bufs)

        def pass_tiles(name, tag, coarse, bufs):
            """Allocate psum tiles for one pass; returns list of (tile, col0)."""
            if coarse:
                t = ps.tile([128, 2048], F32, name=f"{name}A", tag=tag, bufs=bufs)
                return [(t, 0)]
            return [
                (ps.tile([128, 1024], F32, name=f"{name}{h}", tag=tag, bufs=bufs), 1024 * h)
                for h in range(2)
            ]

        def pass_mm2(src_a, src_b, tiles):
            """pass1 of (src_a - src_b): accumulate band*src_a + (-band)*src_b."""
            for pt, col0 in tiles:
                for ci in range(pt.shape[-1] // 512):
                    c = (col0 // 512) + ci
                    for g2, (srcx, bd) in enumerate(
                        ((src_a, band), (src_b, band_neg))
                    ):
                        for k in range(NB):
                            lo, hi = COL_RANGES[k]
                            nc.tensor.matmul(
                                pt[:, 512 * ci + lo : 512 * ci + hi],
                                srcx[:, 512 * k + 128 * c : 512 * k + 128 * c + 128],
                                bd[k][:, : hi - lo],
                                start=(g2 == 0 and k == 0),
                                stop=(g2 == 1 and k == 3),
                            )

        def pass_mm(src, tiles):
            """Band matmuls: src f16 [128,2048] -> psum tiles (raw sums)."""
            for pt, col0 in tiles:
                for ci in range(pt.shape[-1] // 512):
                    c = (col0 // 512) + ci
                    for k in range(NB):
                        lo, hi = COL_RANGES[k]
                        nc.tensor.matmul(
                            pt[:, 512 * ci + lo : 512 * ci + hi],
                            src[:, 512 * k + 128 * c : 512 * k + 128 * c + 128],
                            band[k][:, : hi - lo],
                            start=(k == 0),
                            stop=(k == 3),
                        )

        warm = ps.tile([128, 1024], F32, name="warm", tag="pA", bufs=2)
        for wi in range(16):
            nc.tensor.matmul(
                warm[:, 0:144], bandt[:, 0:128], bandt[:, 0:144],
                start=True, stop=True,
            )

        st = [dict() for _ in range(n_img)]

        # ---- stage 1 both images ----
        for img in range(n_img):
            sfx = f"i{img}"
            xn = []
            for ch in range(3):
                xin = plane_tile(f"xin{sfx}{ch}", "xin", 2)
                nc.sync.dma_start(xin[:], dram_plane_ap(x_ext, img, ch))
                if img == 0 and ch == 0:
                    # band DMA after the first input plane (head-latency)
                    bext = dataclasses.replace(
                        band_ext[0, 0:128, :],
                        ap=mybir.VecI64Pair([[144, 128], [128 * 144, NB], [1, 144]]),
                    )
                    nc.sync.dma_start(bandt[:], bext)
                    nc.vector.tensor_scalar(bandn[:], bandt[:], -1.0, None,
                                            ALU.mult)
                TS(xin[:], xin[:],
                   float(IMAGENET_STD[ch]), float(IMAGENET_MEAN[ch]),
                   ALU.mult, ALU.add)
                xnc = plane_tile(f"xn{sfx}{ch}", "xn", 6)
                TS(xnc[:], xin[:], 0.0, 1.0, ALU.max, ALU.min)
                xn.append(xnc)
            gray3 = plane_tile(f"gray3{sfx}", "gray3", 2)
            TT(gray3[:], xn[0][:], xn[1][:], ALU.add)
            TT(gray3[:], gray3[:], xn[2][:], ALU.add)
            st[img].update(xn=xn, gray3=gray3, mg=[None] * 3, geps=[None] * 3,
                           cov=[None] * 3, a=[None] * 3, b=[None] * 3,
                           q=[None] * 3, s=[None] * 3, nsq=[None] * 3,
                           u=[None] * 3, den=[None] * 3, w=[None] * 3)

        def src_of(img, kind, ch):
            S = st[img]
            sfx = f"i{img}{ch}"
            if kind == "g":
                return S["xn"][ch]
            if kind == "gg":
                t = plane_tile(f"gg{sfx}", "prod", 2)
                nc.gpsimd.tensor_mul(t[:], S["xn"][ch][:], S["xn"][ch][:])
                # u_c = (mi3/3)*mg_c on Pool (slack until gi drain)
                u = plane_tile(f"u{img}{ch}", "u", 3)
                TT(u[:], S["mi"][:], S["mg"][ch][:], ALU.mult)
                S["u"][ch] = u
                nsq = plane_tile(f"nsq{img}{ch}", "nsq", 3)
                nc.gpsimd.tensor_mul(nsq[:], S["mg"][ch][:], S["mg"][ch][:])
                S["nsq"][ch] = nsq
                return t
            if kind == "gi":
                t = plane_tile(f"gi{sfx}", "prod", 2)
                TT(t[:], S["xn"][ch][:], S["gray3"][:], ALU.mult)
                return t
            if kind == "a":
                return S["a"][ch]
            if kind == "b":
                return (S["mi"], S["w"][ch])
            raise KeyError(kind)

        def drain(img, kind, ch, pb, col0):
            S = st[img]
            sfx = f"i{img}{ch}"
            sl = slice(col0, col0 + pb.shape[-1])
            if kind == "g":
                if S["mg"][ch] is None:
                    S["mg"][ch] = plane_tile(f"mg{sfx}", "mg", 6)
                nc.scalar.activation(S["mg"][ch][:, sl], pb[:], AF.Copy,
                                     scale=1.0 / KK)
            elif kind == "gg":
                if S["geps"][ch] is None:
                    S["geps"][ch] = plane_tile(f"ge{sfx}", "geps", 3)
                nc.scalar.activation(S["geps"][ch][:, sl], pb[:], AF.Copy,
                                     scale=1.0 / KK, bias=EPS)
            elif kind == "gi":
                if S["cov"][ch] is None:
                    S["cov"][ch] = plane_tile(f"cov{sfx}", "cov", 3)
                    S["den"][ch] = plane_tile(f"dn{sfx}", "den", 2)
                    S["a"][ch] = plane_tile(f"a{sfx}", "a", 3)
                    S["w"][ch] = plane_tile(f"w{sfx}", "w", 3)
                cov = S["cov"][ch]
                den = S["den"][ch]
                a = S["a"][ch]
                w = S["w"][ch]
                STT(cov[:, sl], pb[:], 1.0 / (3.0 * KK),
                    S["u"][ch][:, sl], ALU.mult, ALU.subtract)
                TT(den[:, sl], S["geps"][ch][:, sl], S["nsq"][ch][:, sl],
                   ALU.subtract)
                if col0 >= 1024:
                    nc.scalar.activation(den[:], den[:], AF.Ln)
                    nc.scalar.activation(den[:], den[:], AF.Exp, scale=-1.0)
                    for hh in range(2):
                        s2 = slice(1024 * hh, 1024 * (hh + 1))
                        TT(a[:, s2], cov[:, s2], den[:, s2], ALU.mult)
                        nc.gpsimd.tensor_mul(w[:, s2], a[:, s2],
                                             S["mg"][ch][:, s2])
            elif kind == "a":
                if S["q"][ch] is None:
                    S["q"][ch] = plane_tile(f"q{sfx}", "q", 2)
                    S["s"][ch] = plane_tile(f"s{sfx}", "s", 3)
                STT(S["q"][ch][:, sl], pb[:], 1.0 / KK,
                    S["xn"][ch][:, sl], ALU.mult, ALU.mult)
                nc.gpsimd.tensor_sub(S["s"][ch][:, sl], S["xn"][ch][:, sl],
                                     S["q"][ch][:, sl])
            elif kind == "b":
                STT(S["s"][ch][:, sl], pb[:], -1.0 / KK,
                    S["s"][ch][:, sl], ALU.mult, ALU.add)
                h0 = col0 // 1024
                oap = dataclasses.replace(
                    out_ext[img, ch, 256 * h0 : 256 * h0 + 128, :],
                    ap=mybir.VecI64Pair([[512, 128], [65536, 2], [1, 512]]),
                )
                nc.sync.dma_start(oap, S["s"][ch][:, sl])

        def after_drain(img, kind, ch):
            S = st[img]
            sfx = f"i{img}{ch}"
            if kind == "g" and ch == 2:
                mi = plane_tile(f"mi{sfx}", "mi", 2)
                TT(mi[:], S["mg"][0][:], S["mg"][1][:], ALU.add)
                TT(mi[:], mi[:], S["mg"][2][:], ALU.add)
                TS(mi[:], mi[:], 1.0 / 3.0, None, ALU.mult)
                S["mi"] = mi
            elif kind == "gi":
                pass
            elif kind == "a":
                pass
            elif kind == "b":
                pass

        kinds = (
            [("g", c) for c in range(3)]
            + [("gg", 0), ("gi", 0), ("gg", 1), ("a", 0), ("gi", 1),
               ("b", 0), ("gg", 2), ("a", 1), ("gi", 2), ("b", 1),
               ("a", 2), ("b", 2)]
        )
        planes = [(img, kind, ch) for (kind, ch) in kinds for img in range(n_img)]

        # copy engines per plane kind: early phase (g) -> DVE has slack;
        # cluster phase -> ACT
        def copy_engines(idx, kind):
            if kind in ("g",):
                return ("act", "dve")
            if kind == "gg":
                return ("act", "act")
            return ("act", "act")

        def emit_copy(vt, pa_tiles, engs):
            for (pt, col0), eng in zip(pa_tiles, engs):
                dst = vt[:, col0 : col0 + pt.shape[-1]]
                if eng == "act":
                    nc.scalar.activation(dst, pt[:], AF.Copy)
                else:
                    nc.vector.tensor_copy(dst, pt[:])

        vt_of = {}
        prev = None
        pa_bufs = 1 if PA_COARSE else 2
        pb_bufs = 1 if PB_COARSE else 2
        for idx, (img, kind, ch) in enumerate(planes):
            name = f"{kind}{img}{ch}"
            src = src_of(img, kind, ch)
            pa = pass_tiles(f"pa_{name}", "pA", PA_COARSE, pa_bufs)
            if kind == "b":
                pass_mm2(src[0], src[1], pa)
            else:
                pass_mm(src, pa)
            vt = plane_tile(f"vt_{name}", "vt", 3)
            emit_copy(vt, pa, copy_engines(idx, kind))
            vt_of[(img, kind, ch)] = vt

            if prev is not None:
                pimg, pkind, pch = prev
                pvt = vt_of.pop(prev)
                pb = pass_tiles(f"pb_{pkind}{pimg}{pch}", "pB", PB_COARSE, pb_bufs)
                pass_mm(pvt, pb)
                for pt, col0 in pb:
                    drain(pimg, pkind, pch, pt, col0)
                after_drain(pimg, pkind, pch)
            prev = (img, kind, ch)

        pimg, pkind, pch = prev
        pvt = vt_of.pop(prev)
        pb = pass_tiles(f"pb_{pkind}{pimg}{pch}", "pB", PB_COARSE, pb_bufs)
        pass_mm(pvt, pb)
        for pt, col0 in pb:
            drain(pimg, pkind, pch, pt, col0)
        after_drain(pimg, pkind, pch)

    fixup_waits(nc)
    return nc


_CACHED = {}


def _get_nc():
    if "nc" not in _CACHED:
        _CACHED["nc"] = build_core_kernel()
    return _CACHED["nc"]


def kernel(x: np.ndarray) -> np.ndarray:
    from concourse.bass_utils import run_bass_kernel_spmd

    assert x.shape == (16, 3, 512, 512)
    band = band_consts()
    x16 = x.astype(np.float16)
    nc = _get_nc()
    in_maps = [
        {
            "x": np.ascontiguousarray(x16[2 * i : 2 * i + 2]),
            "band": band,
        }
        for i in range(8)
    ]
    res = run_bass_kernel_spmd(nc, in_maps, core_ids=list(range(8)))
    return np.concatenate(
        [np.asarray(r["out"]).astype(np.float32) for r in res.results], axis=0
    )


if __name__ == "__main__":
    x = np.random.default_rng(0).standard_normal((16, 3, 512, 512)).astype(np.float32)
    y = kernel(x)
    print(y.shape, y.dtype, float(np.abs(y).max()))


# revision 10
# speedup vs baseline: 1.1222x; 1.0448x over previous
"""DetailBranch (guided-filter detail extraction) Trainium2 Bass kernel.

r=8 -> 17x17 zero-padded box mean, eps=1e-3.  Math per image (f16 data
planes, f32 psum accumulate):
  xn    = clip(x*std+mean, 0, 1)
  gray3 = xn0+xn1+xn2 ; mi = (mg0+mg1+mg2)/3
  mg_c  = box(xn_c)
  cov_c = box(xn_c*gray3)/867 - mg_c*mi      (= cov(g, gray))
  den_c = box(xn_c^2)/289 + eps - mg_c^2     (= var + eps)
  a_c   = cov_c * exp(-ln(den_c)) ; b_c = mi - a_c*mg_c
  out_c = xn_c - box(a_c)*xn_c - box(b_c)

Box = two TensorE band-matmul passes; each filters the partition dim and
transposes (two passes -> upright).  The moving operand is a unit-valued
f16 band matrix (exact; 1 cycle/row), the stationary operand is the f16
data plane; all 1/289-style normalizations are folded into the
psum-draining elementwise ops as exact f32 scalars.

15 box planes per image (g,gi,gg,a,b x3 channels), both images
interleaved plane-by-plane, per-channel clusters staggered
(gg0,gi0,gg1,a0,gi1,b0,...) so each stage-3 chain has pipeline slack.
Stream is software-pipelined with stagger 1 over two psum rings
(pass1: 2x[128,1024], pass2: 2x[128,1024] = all 8 banks).

Engine placement (GPSIMD cannot access PSUM; TensorScalarPtr and
tensor-tensor-divide are invalid on Pool/DVE respectively, hence
exp(-ln) on ACT for the reciprocal):
  PE   band matmuls (f16, cost ~ moving rows only)
  DVE  2x-mode f16 tensor_tensor chain ops + fused psum stt drains
       (cov/q/o), some pass1 copies
  ACT  psum scale/bias drains (mg/geps), Ln/Exp, most pass1 copies
  Pool SBUF-only muls/subs (gg, nsq, w, b)
I/O is f16 (host casts f32<->f16); one DMA per input plane, per-half
output DMAs via 3-d DRAM access patterns.  16 dummy band matmuls warm
the PE p-state during the stage-1 DMA latency (full clock needs ~3us of
continuous PE busy).

Sharding: pure batch data-parallel, 2 images per core on 8 cores.
"""

import sys

sys.path.insert(0, "/opt/trn_rl_repo")

import contextlib
import dataclasses

import numpy as np

import concourse.bass as bass
import concourse.mybir as mybir
import concourse.tile as tile

from bass_rust import SyncInfo


EXEMPT = {"InstNoOp", "InstEventSemaphore", "InstAllEngineBarrier",
          "InstSemaphoreOp", "InstHalt"}


def fixup_waits(nc, verbose=False):
    for fn in nc.m.functions:
        targets = []
        for blk in fn.blocks:
            for inst in blk.instructions:
                if (
                    type(inst).__name__ not in EXEMPT
                    and inst.sync_info is not None
                    and len(inst.sync_info.on_wait) > 1
                ):
                    targets.append((blk, inst.name, inst.engine, 1))
        if not targets:
            continue
        for k, (blk, tname, eng, lim) in enumerate(targets):
            il = blk.instructions
            idx = next(j for j, x in enumerate(il) if x.name == tname)
            inst = il[idx]
            si = inst.sync_info
            waits = list(si.on_wait)
            evs = [
                mybir.InstEventSemaphore(
                    name=f"EVW{k}-{j}-{tname}", engine=eng, ins=[], outs=[],
                    sync_info=SyncInfo(on_wait=[w], on_update=[]),
                    bass_nofuse=True,
                )
                for j, w in enumerate(waits[:-lim])
            ]
            inst.sync_info = SyncInfo(
                on_wait=waits[-lim:], on_update=list(si.on_update)
            )
            il[idx:idx] = evs
            if verbose:
                print(f"fixup: {tname}({eng}) {len(waits)} waits -> {len(evs)} evsems")
    return nc


R = 8
KK = float((2 * R + 1) ** 2)  # 289
EPS = 1e-3
H = W = 512
NB = 4
F32 = mybir.dt.float32
F16 = mybir.dt.float16
AF = mybir.ActivationFunctionType
ALU = mybir.AluOpType
IMAGENET_MEAN = [0.485, 0.456, 0.406]
IMAGENET_STD = [0.229, 0.224, 0.225]

COL_RANGES = [(max(0, 128 * k - 8), min(512, 128 * k + 136)) for k in range(NB)]

# psum granularity knobs: True = one [128,2048] tile per pass (1 copy/drain
# op), False = two [128,1024] tiles (2 ops, deeper ring)
PA_COARSE = False
PB_COARSE = False


def band_consts():
    i = np.arange(512)
    B = (np.abs(i[:, None] - i[None, :]) <= R).astype(np.float16)
    band1 = np.zeros((NB, 128, 144), np.float16)
    for k, (lo, hi) in enumerate(COL_RANGES):
        band1[k, :, : hi - lo] = B[128 * k : 128 * k + 128, lo:hi]
    return band1


def build_core_kernel(n_img=2):
    nc = bass.Bass()
    x_ext = nc.dram_tensor("x", [n_img, 3, H, W], F16, kind="ExternalInput")
    band_ext = nc.dram_tensor("band", [NB, 128, 144], F16, kind="ExternalInput")
    out_ext = nc.dram_tensor("out", [n_img, 3, H, W], F16, kind="ExternalOutput")

    with contextlib.ExitStack() as ctx:
        tc = ctx.enter_context(tile.TileContext(nc))
        const = ctx.enter_context(tc.tile_pool(name="const", bufs=1))
        sb = ctx.enter_context(tc.tile_pool(name="sb", bufs=1))
        ps = ctx.enter_context(tc.tile_pool(name="ps", bufs=1, space="PSUM"))

        def dram_plane_ap(ext, img, ch, nb=NB):
            ap = ext[img, ch, 0:128, :]
            return dataclasses.replace(
                ap, ap=mybir.VecI64Pair([[512, 128], [65536, nb], [1, 512]])
            )

        bandt = const.tile([128, NB * 144], F16, name="band")
        band = [bandt[:, 144 * k : 144 * (k + 1)] for k in range(NB)]

        bandn = const.tile([128, NB * 144], F16, name="bandn")
        band_neg = [bandn[:, 144 * k : 144 * (k + 1)] for k in range(NB)]

        TT = nc.vector.tensor_tensor
        TS = nc.vector.tensor_scalar
        STT = nc.vector.scalar_tensor_tensor
        PSTT = nc.gpsimd.scalar_tensor_tensor

        def plane_tile(name, tag, bufs):
            return sb.tile([128, 2048], F16, name=name, tag=tag, bufs=bufs)

        def pass_tiles(name, tag, coarse, bufs):
            """Allocate psum tiles for one pass; returns list of (tile, col0)."""
            if coarse:
                t = ps.tile([128, 2048], F32, name=f"{name}A", tag=tag, bufs=bufs)
                return [(t, 0)]
            return [
                (ps.tile([128, 1024], F32, name=f"{name}{h}", tag=tag, bufs=bufs), 1024 * h)
                for h in range(2)
            ]

        def pass_mm2(src_a, src_b, tiles):
            """pass1 of (src_a - src_b): accumulate band*src_a + (-band)*src_b."""
            for pt, col0 in tiles:
                for ci in range(pt.shape[-1] // 512):
                    c = (col0 // 512) + ci
                    for g2, (srcx, bd) in enumerate(
                        ((src_a, band), (src_b, band_neg))
                    ):
                        for k in range(NB):
                            lo, hi = COL_RANGES[k]
                            nc.tensor.matmul(
                                pt[:, 512 * ci + lo : 512 * ci + hi],
                                srcx[:, 512 * k + 128 * c : 512 * k + 128 * c + 128],
                                bd[k][:, : hi - lo],
                                start=(g2 == 0 and k == 0),
                                stop=(g2 == 1 and k == 3),
                            )

        def pass_mm(src, tiles):
            """Band matmuls: src f16 [128,2048] -> psum tiles (raw sums)."""
            for pt, col0 in tiles:
                for ci in range(pt.shape[-1] // 512):
                    c = (col0 // 512) + ci
                    for k in range(NB):
                        lo, hi = COL_RANGES[k]
                        nc.tensor.matmul(
                            pt[:, 512 * ci + lo : 512 * ci + hi],
                            src[:, 512 * k + 128 * c : 512 * k + 128 * c + 128],
                            band[k][:, : hi - lo],
                            start=(k == 0),
                            stop=(k == 3),
                        )

        warm = ps.tile([128, 1024], F32, name="warm", tag="pA", bufs=2)
        for wi in range(16):
            nc.tensor.matmul(
                warm[:, 0:144], bandt[:, 0:128], bandt[:, 0:144],
                start=True, stop=True,
            )

        st = [dict() for _ in range(n_img)]

        # ---- stage 1 both images ----
        for img in range(n_img):
            sfx = f"i{img}"
            xn = []
            for ch in range(3):
                xin = plane_tile(f"xin{sfx}{ch}", "xin", 2)
                nc.sync.dma_start(xin[:], dram_plane_ap(x_ext, img, ch))
                if img == 0 and ch == 0:
                    # band DMA after the first input plane (head-latency)
                    bext = dataclasses.replace(
                        band_ext[0, 0:128, :],
                        ap=mybir.VecI64Pair([[144, 128], [128 * 144, NB], [1, 144]]),
                    )
                    nc.sync.dma_start(bandt[:], bext)
                    nc.vector.tensor_scalar(bandn[:], bandt[:], -1.0, None,
                                            ALU.mult)
                TS(xin[:], xin[:],
                   float(IMAGENET_STD[ch]), float(IMAGENET_MEAN[ch]),
                   ALU.mult, ALU.add)
                xnc = plane_tile(f"xn{sfx}{ch}", "xn", 6)
                TS(xnc[:], xin[:], 0.0, 1.0, ALU.max, ALU.min)
                xn.append(xnc)
            gray3 = plane_tile(f"gray3{sfx}", "gray3", 2)
            TT(gray3[:], xn[0][:], xn[1][:], ALU.add)
            TT(gray3[:], gray3[:], xn[2][:], ALU.add)
            st[img].update(xn=xn, gray3=gray3, mg=[None] * 3, geps=[None] * 3,
                           cov=[None] * 3, a=[None] * 3, b=[None] * 3,
                           q=[None] * 3, s=[None] * 3, nsq=[None] * 3,
                           u=[None] * 3, den=[None] * 3, w=[None] * 3)

        def src_of(img, kind, ch):
            S = st[img]
            sfx = f"i{img}{ch}"
            if kind == "g":
                return S["xn"][ch]
            if kind == "gg":
                t = plane_tile(f"gg{sfx}", "prod", 2)
                nc.gpsimd.tensor_mul(t[:], S["xn"][ch][:], S["xn"][ch][:])
                # u_c = (mi3/3)*mg_c on Pool (slack until gi drain)
                u = plane_tile(f"u{img}{ch}", "u", 3)
                TT(u[:], S["mi"][:], S["mg"][ch][:], ALU.mult)
                S["u"][ch] = u
                nsq = plane_tile(f"nsq{img}{ch}", "nsq", 3)
                nc.gpsimd.tensor_mul(nsq[:], S["mg"][ch][:], S["mg"][ch][:])
                S["nsq"][ch] = nsq
                return t
            if kind == "gi":
                t = plane_tile(f"gi{sfx}", "prod", 2)
                TT(t[:], S["xn"][ch][:], S["gray3"][:], ALU.mult)
                return t
            if kind == "a":
                return S["a"][ch]
            if kind == "b":
                return (S["mi"], S["w"][ch])
            raise KeyError(kind)

        def drain(img, kind, ch, pb, col0):
            S = st[img]
            sfx = f"i{img}{ch}"
            sl = slice(col0, col0 + pb.shape[-1])
            if kind == "g":
                if S["mg"][ch] is None:
                    S["mg"][ch] = plane_tile(f"mg{sfx}", "mg", 6)
                nc.scalar.activation(S["mg"][ch][:, sl], pb[:], AF.Copy,
                                     scale=1.0 / KK)
            elif kind == "gg":
                if S["geps"][ch] is None:
                    S["geps"][ch] = plane_tile(f"ge{sfx}", "geps", 3)
                if img == 1:
                    TS(S["geps"][ch][:, sl], pb[:], 1.0 / KK, EPS,
                       ALU.mult, ALU.add)
                else:
                    nc.scalar.activation(S["geps"][ch][:, sl], pb[:], AF.Copy,
                                         scale=1.0 / KK, bias=EPS)
            elif kind == "gi":
                if S["cov"][ch] is None:
                    S["cov"][ch] = plane_tile(f"cov{sfx}", "cov", 3)
                    S["den"][ch] = plane_tile(f"dn{sfx}", "den", 2)
                    S["a"][ch] = plane_tile(f"a{sfx}", "a", 3)
                    S["w"][ch] = plane_tile(f"w{sfx}", "w", 3)
                cov = S["cov"][ch]
                den = S["den"][ch]
                a = S["a"][ch]
                w = S["w"][ch]
                STT(cov[:, sl], pb[:], 1.0 / (3.0 * KK),
                    S["u"][ch][:, sl], ALU.mult, ALU.subtract)
                TT(den[:, sl], S["geps"][ch][:, sl], S["nsq"][ch][:, sl],
                   ALU.subtract)
                if col0 >= 1024:
                    nc.scalar.activation(den[:], den[:], AF.Ln)
                    nc.scalar.activation(den[:], den[:], AF.Exp, scale=-1.0)
                    for hh in range(2):
                        s2 = slice(1024 * hh, 1024 * (hh + 1))
                        TT(a[:, s2], cov[:, s2], den[:, s2], ALU.mult)
                        nc.gpsimd.tensor_mul(w[:, s2], a[:, s2],
                                             S["mg"][ch][:, s2])
            elif kind == "a":
                if S["q"][ch] is None:
                    S["q"][ch] = plane_tile(f"q{sfx}", "q", 2)
                    S["s"][ch] = plane_tile(f"s{sfx}", "s", 3)
                STT(S["q"][ch][:, sl], pb[:], 1.0 / KK,
                    S["xn"][ch][:, sl], ALU.mult, ALU.mult)
                nc.gpsimd.tensor_sub(S["s"][ch][:, sl], S["xn"][ch][:, sl],
                                     S["q"][ch][:, sl])
            elif kind == "b":
                STT(S["s"][ch][:, sl], pb[:], -1.0 / KK,
                    S["s"][ch][:, sl], ALU.mult, ALU.add)
                h0 = col0 // 1024
                oap = dataclasses.replace(
                    out_ext[img, ch, 256 * h0 : 256 * h0 + 128, :],
                    ap=mybir.VecI64Pair([[512, 128], [65536, 2], [1, 512]]),
                )
                nc.sync.dma_start(oap, S["s"][ch][:, sl])

        def after_drain(img, kind, ch):
            S = st[img]
            sfx = f"i{img}{ch}"
            if kind == "g" and ch == 2:
                mi = plane_tile(f"mi{sfx}", "mi", 2)
                TT(mi[:], S["mg"][0][:], S["mg"][1][:], ALU.add)
                TT(mi[:], mi[:], S["mg"][2][:], ALU.add)
                TS(mi[:], mi[:], 1.0 / 3.0, None, ALU.mult)
                S["mi"] = mi
            elif kind == "gi":
                pass
            elif kind == "a":
                pass
            elif kind == "b":
                pass

        kinds = (
            [("g", c) for c in range(3)]
            + [("gg", 0), ("gi", 0), ("gg", 1), ("gi", 1), ("a", 0),
               ("gg", 2), ("b", 0), ("a", 1), ("gi", 2), ("b", 1),
               ("a", 2), ("b", 2)]
        )
        planes = [(img, kind, ch) for (kind, ch) in kinds for img in range(n_img)]

        # copy engines per plane kind: early phase (g) -> DVE has slack;
        # cluster phase -> ACT
        def copy_engines(idx, kind):
            if kind in ("g",):
                return ("act", "dve")
            if kind == "gg":
                return ("act", "act")
            return ("act", "act")

        def emit_copy(vt, pa_tiles, engs):
            for (pt, col0), eng in zip(pa_tiles, engs):
                dst = vt[:, col0 : col0 + pt.shape[-1]]
                if eng == "act":
                    nc.scalar.activation(dst, pt[:], AF.Copy)
                else:
                    nc.vector.tensor_copy(dst, pt[:])

        vt_of = {}
        prev = None
        pa_bufs = 1 if PA_COARSE else 2
        pb_bufs = 1 if PB_COARSE else 2
        for idx, (img, kind, ch) in enumerate(planes):
            name = f"{kind}{img}{ch}"
            src = src_of(img, kind, ch)
            pa = pass_tiles(f"pa_{name}", "pA", PA_COARSE, pa_bufs)
            if kind == "b":
                pass_mm2(src[0], src[1], pa)
            else:
                pass_mm(src, pa)
            vt = plane_tile(f"vt_{name}", "vt", 3)
            emit_copy(vt, pa, copy_engines(idx, kind))
            vt_of[(img, kind, ch)] = vt

            if prev is not None:
                pimg, pkind, pch = prev
                pvt = vt_of.pop(prev)
                pb = pass_tiles(f"pb_{pkind}{pimg}{pch}", "pB", PB_COARSE, pb_bufs)
                pass_mm(pvt, pb)
                for pt, col0 in pb:
                    drain(pimg, pkind, pch, pt, col0)
                after_drain(pimg, pkind, pch)
            prev = (img, kind, ch)

        pimg, pkind, pch = prev
        pvt = vt_of.pop(prev)
        pb = pass_tiles(f"pb_{pkind}{pimg}{pch}", "pB", PB_COARSE, pb_bufs)
        pass_mm(pvt, pb)
        for pt, col0 in pb:
            drain(pimg, pkind, pch, pt, col0)
        after_drain(pimg, pkind, pch)

    fixup_waits(nc)
    return nc


_CACHED = {}


def _get_nc():
    if "nc" not in _CACHED:
        _CACHED["nc"] = build_core_kernel()
    return _CACHED["nc"]


def kernel(x: np.ndarray) -> np.ndarray:
    from concourse.bass_utils import run_bass_kernel_spmd

    assert x.shape == (16, 3, 512, 512)
    band = band_consts()
    x16 = x.astype(np.float16)
    nc = _get_nc()
    in_maps = [
        {
            "x": np.ascontiguousarray(x16[2 * i : 2 * i + 2]),
            "band": band,
        }
        for i in range(8)
    ]
    res = run_bass_kernel_spmd(nc, in_maps, core_ids=list(range(8)))
    return np.concatenate(
        [np.asarray(r["out"]).astype(np.float32) for r in res.results], axis=0
    )


if __name__ == "__main__":
    x = np.random.default_rng(0).standard_normal((16, 3, 512, 512)).astype(np.float32)
    y = kernel(x)
    print(y.shape, y.dtype, float(np.abs(y).max()))
